# revision 1
# baseline (speedup 1.0000x reference)
"""Causal self-attention TRN2 kernel.

Problem: B=4, L=2048, D=768, H=6 heads, head_dim=128, fp32, causal mask
tril(k=1) (each query row q attends to keys k <= q+1).

Sharding: 8 cores = 4 batches x 2 head-groups (3 heads each).
Each core computes, for its batch b and heads [3g, 3g+3):
    Q = x_b @ Wq[:, cols] + bq[cols]   (and K, V likewise)
    per head: S^T = K @ Q^T (scaled), P = exp(S) masked, O = P@V / rowsum
    y_core = (O_heads @ Wo[rows, :])^T          -> [768, 2048] partial
Host: out[b] = (y[2b] + y[2b+1])^T + bo + bv @ Wo   (attn rows sum to 1,
so the V bias contributes exactly bv @ Wo_rows to every output row).

Layout trick: everything is kept transposed (feature dim on partitions) so
every matmul has a 512-wide moving operand and can run at full PE rate in
float32r (x itself arrives host-transposed, so no on-chip transposes at
all). Softmax runs without max-subtraction (logits are O(1) here), with
row sums computed by a ones-vector matmul in the same transposed layout,
then broadcast via a rank-1 matmul for the normalization multiply.
Projections run one 512-column chunk ahead of attention because the
tril(k=1) mask lets each query attend one token into the future.
Diagonal-band blocks restrict S/exp/mask/PV/rowsum to the valid column
range (everything below 128j-2 is structurally masked), cutting ~12% of
PE work.
"""

import math
from contextlib import ExitStack

import numpy as np

import concourse.tile as tile
from concourse import bacc, mybir
from concourse.bass_utils import run_bass_kernel_spmd

F32 = mybir.dt.float32
F32R = mybir.dt.float32r
AF = mybir.ActivationFunctionType

B, L, D, H = 4, 2048, 768, 6
HD = 128           # head dim
HPC = 3            # heads per core
DH = HPC * HD      # 384: per-core projection width
NCORES = 8
P = 128
CHUNK = 512        # q-chunk width (moving-operand size)
NCHUNK = L // CHUNK
LT = L // P        # 16 L-tiles
DT = D // P        # 6 d-tiles
SCALE = 1.0 / math.sqrt(HD)

_cache = {}


def build_nc(mm_fast=True, enable_asserts=False, reps=1,
             mm1_bufs=3, aux_bufs=1, pb_in_mm1=False, den_on_dve=False,
             est_bufs=6, attn_mode="seq", mask_gpsimd=False, vt_on_act=False,
             fused_denb=True, qk_on_dve=False, yst_on_act=False,
             attn_order="pipeline", qt_bufs=2, denom_mode="mm"):
    nc = bacc.Bacc(
        "TRN2",
        target_bir_lowering=False,
        debug=False,
        enable_asserts=enable_asserts,
        num_devices=NCORES,
    )
    x_d = nc.dram_tensor("x", [D, L], F32, kind="ExternalInput").ap()
    wq_d = nc.dram_tensor("wq", [D, DH], F32, kind="ExternalInput").ap()
    wk_d = nc.dram_tensor("wk", [D, DH], F32, kind="ExternalInput").ap()
    wv_d = nc.dram_tensor("wv", [D, DH], F32, kind="ExternalInput").ap()
    wo_d = nc.dram_tensor("wo", [DH, D], F32, kind="ExternalInput").ap()
    bq_d = nc.dram_tensor("bq", [DH], F32, kind="ExternalInput").ap()
    bk_d = nc.dram_tensor("bk", [DH], F32, kind="ExternalInput").ap()
    y_d = nc.dram_tensor("y", [D, L], F32, kind="ExternalOutput").ap()

    MMDT = F32R if mm_fast else F32
    cast = lambda ap: ap  # noqa: E731

    with tile.TileContext(nc) as tc, ExitStack() as ctx:
        const = ctx.enter_context(tc.tile_pool(name="const", bufs=1))
        wpool = ctx.enter_context(tc.tile_pool(name="wts", bufs=1))
        kvpool = ctx.enter_context(tc.tile_pool(name="kv", bufs=1))
        xtpool = ctx.enter_context(tc.tile_pool(name="xt", bufs=2))
        qpool = ctx.enter_context(tc.tile_pool(name="qt", bufs=qt_bufs))
        estpool = ctx.enter_context(tc.tile_pool(name="est", bufs=est_bufs))
        smpool = ctx.enter_context(tc.tile_pool(name="sm", bufs=2))
        opool = ctx.enter_context(tc.tile_pool(name="ot", bufs=2))
        espool = ctx.enter_context(tc.tile_pool(name="esum", bufs=2))
        ypool = ctx.enter_context(tc.tile_pool(name="yst", bufs=3))
        ps_mm = ctx.enter_context(tc.tile_pool(name="psmm", bufs=mm1_bufs, space="PSUM"))
        ps_acc = ctx.enter_context(tc.tile_pool(name="psacc", bufs=2, space="PSUM"))
        ps_aux = ctx.enter_context(tc.tile_pool(name="psaux", bufs=aux_bufs, space="PSUM"))

        ones_f32 = const.tile([P, 1], F32, tag="ones_f32", name="ones_f32")
        nc.vector.memset(ones_f32[:], 1.0)
        ones_col = const.tile([P, 1], MMDT, tag="ones_col", name="ones_col")
        nc.scalar.copy(ones_col[:], ones_f32[:])
        ones_mf = const.tile([P, P], F32, tag="ones_mf", name="ones_mf")
        nc.vector.memset(ones_mf[:], 1.0)
        ones_mat = const.tile([P, P], MMDT, tag="ones_mat", name="ones_mat")
        nc.scalar.copy(ones_mat[:], ones_mf[:])
        ones_rf = const.tile([1, P], F32, tag="ones_rf", name="ones_rf")
        nc.vector.memset(ones_rf[:], 1.0)
        ones_row = const.tile([1, P], MMDT, tag="ones_row", name="ones_row")
        nc.scalar.copy(ones_row[:], ones_rf[:])
        # 5 diagonal-band masks (0/1), shared by all chunks/heads.
        # mask[j][kp, qq] = 1 iff kp - qq <= 1 - 128*j
        masks = []
        for j in range(5):
            mj = const.tile([P, CHUNK], F32, tag=f"mask{j}", name=f"mask{j}")
            nc.gpsimd.memset(mj[:], 1.0)
            # keep (mask=1) where kp - qq <= 1 - 128*j, i.e. qq - kp + (1-128j) >= 0
            nc.gpsimd.affine_select(
                out=mj[:],
                in_=mj[:],
                pattern=[[1, CHUNK]],
                compare_op=mybir.AluOpType.is_ge,
                fill=0.0,
                base=1 - 128 * j,
                channel_multiplier=-1,
            )
            masks.append(mj)

        def make_xT(c):
            # xT columns: block d lives at [d*CHUNK, (d+1)*CHUNK)
            xT = xtpool.tile([P, DT * CHUNK], MMDT, tag="xT", name="xT")
            for d in range(DT):
                nc.sync.dma_start(
                    out=xT[:, d * CHUNK:(d + 1) * CHUNK],
                    in_=x_d[d * P:(d + 1) * P,
                            c * CHUNK:(c + 1) * CHUNK].bitcast(MMDT),
                )
            return xT

        # DMA issue order: wq tiles and chunk-0 x columns first so the first
        # projection matmuls can start early; wk/wv next; wo/biases later.
        wq = []
        for d in range(DT):
            wq_t = wpool.tile([P, DH], MMDT, tag=f"wq{d}", name=f"wq{d}")
            nc.sync.dma_start(out=wq_t[:], in_=wq_d[d * P:(d + 1) * P, :].bitcast(MMDT))
            wq.append(wq_t)
        xT0 = make_xT(0)
        wk = []
        wv = []
        for d in range(DT):
            wk_t = wpool.tile([P, DH], MMDT, tag=f"wk{d}", name=f"wk{d}")
            nc.sync.dma_start(out=wk_t[:], in_=wk_d[d * P:(d + 1) * P, :].bitcast(MMDT))
            wk.append(wk_t)
            wv_t = wpool.tile([P, DH], MMDT, tag=f"wv{d}", name=f"wv{d}")
            nc.sync.dma_start(out=wv_t[:], in_=wv_d[d * P:(d + 1) * P, :].bitcast(MMDT))
            wv.append(wv_t)
        bq_t = []
        bk_t = []
        for h in range(HPC):
            bq_h = wpool.tile([P, 1], F32, tag=f"bq{h}", name=f"bq{h}")
            nc.sync.dma_start(
                out=bq_h[:], in_=bq_d[h * P:(h + 1) * P].rearrange("(p o) -> p o", o=1)
            )
            bq_t.append(bq_h)
            bk_h = wpool.tile([P, 1], F32, tag=f"bk{h}", name=f"bk{h}")
            nc.sync.dma_start(
                out=bk_h[:], in_=bk_d[h * P:(h + 1) * P].rearrange("(p o) -> p o", o=1)
            )
            bk_t.append(bk_h)
        wo = []
        for h in range(HPC):
            wo_t = wpool.tile([P, D], MMDT, tag=f"wo{h}", name=f"wo{h}")
            nc.sync.dma_start(out=wo_t[:], in_=wo_d[h * P:(h + 1) * P, :].bitcast(MMDT))
            wo.append(wo_t)

        # K^T per head [hd=128, L]; V per L-tile [kpos=128, 3*hd]
        kT = [kvpool.tile([P, L], MMDT, tag=f"kT{h}", name=f"kT{h}") for h in range(HPC)]
        vt = [kvpool.tile([P, DH], MMDT, tag=f"v{t}", name=f"v{t}") for t in range(LT)]

        def proj_chunk(c, xT=None):
            # ---- x^T columns for this chunk (x arrives host-transposed) ----
            if xT is None:
                xT = make_xT(c)

            # ---- Q^T, K^T projections for this chunk ----
            qT = [qpool.tile([P, CHUNK], MMDT, tag=f"qT{h}", name=f"qT{h}")
                  for h in range(HPC)]
            for h in range(HPC):
                pq = ps_acc.tile([P, CHUNK], F32, tag="acc", name="acc")
                for d in range(DT):
                    nc.tensor.matmul(
                        pq[:],
                        cast(wq[d][:, h * P:(h + 1) * P]),
                        cast(xT[:, d * CHUNK:(d + 1) * CHUNK]),
                        start=(d == 0),
                        stop=(d == DT - 1),
                    )
                if qk_on_dve:
                    nc.vector.tensor_scalar_add(qT[h][:], pq[:], bq_t[h][:])
                else:
                    nc.scalar.activation(qT[h][:], pq[:], AF.Identity,
                                         bias=bq_t[h][:])
                pk = ps_acc.tile([P, CHUNK], F32, tag="acc", name="acc")
                for d in range(DT):
                    nc.tensor.matmul(
                        pk[:],
                        cast(wk[d][:, h * P:(h + 1) * P]),
                        cast(xT[:, d * CHUNK:(d + 1) * CHUNK]),
                        start=(d == 0),
                        stop=(d == DT - 1),
                    )
                if qk_on_dve:
                    nc.vector.tensor_scalar_add(
                        kT[h][:, c * CHUNK:(c + 1) * CHUNK], pk[:], bk_t[h][:]
                    )
                else:
                    nc.scalar.activation(
                        kT[h][:, c * CHUNK:(c + 1) * CHUNK], pk[:], AF.Identity,
                        bias=bk_t[h][:],
                    )

            # ---- V projection (natural layout) ----
            for i in range(CHUNK // P):
                t = c * (CHUNK // P) + i
                pv = ps_acc.tile([P, DH], F32, tag="acc", name="acc")
                for d in range(DT):
                    nc.tensor.matmul(
                        pv[:],
                        cast(xT[:, d * CHUNK + i * P: d * CHUNK + (i + 1) * P]),
                        cast(wv[d][:]),
                        start=(d == 0),
                        stop=(d == DT - 1),
                    )
                if vt_on_act:
                    nc.scalar.copy(vt[t][:], pv[:])
                else:
                    nc.vector.tensor_copy(vt[t][:], pv[:])
            return qT

        def attn_chunk_ileave(c, qT):
            # ---- attention, 3 heads interleaved per kb block ----
            # Denominators for all heads pack into one PSUM bank (rows
            # 0/32/64 -- tile_position requires 32-aligned output rows).
            # A single start=True (h0,kb0) clears the bank; the other heads'
            # first writes overwrite via the has_written bits.
            KB = 4 * c + 5 if c < NCHUNK - 1 else LT
            oTn = [opool.tile([P, CHUNK], MMDT, tag=f"oT{h}", name=f"oT{h}")
                   for h in range(HPC)]
            po = [ps_acc.tile([P, CHUNK], F32, tag=f"pv{h}", name=f"pv{h}", bufs=1)
                  for h in range(HPC)]
            pdall = ps_aux.tile([P, CHUNK], F32, tag="aux", name="aux", bufs=1)
            nc.vector.memset(pdall[:], 0.0)
            for kb in range(KB):
                j = kb - 4 * c
                for h in range(HPC):
                    pst = ps_mm.tile([P, CHUNK], F32, tag="mm1", name="mm1")
                    nc.tensor.matmul(
                        pst[:],
                        cast(kT[h][:, kb * P:(kb + 1) * P]),
                        cast(qT[h][:]),
                        start=True,
                        stop=True,
                    )
                    est = estpool.tile([P, CHUNK], MMDT, tag="est", name="est")
                    nc.scalar.activation(est[:], pst[:], AF.Exp, scale=SCALE)
                    if j >= 0:
                        nc.vector.tensor_mul(est[:], est[:], masks[j][:])
                    nc.tensor.matmul(
                        po[h][:],
                        cast(vt[kb][:, h * P:(h + 1) * P]),
                        cast(est[:]),
                        start=(kb == 0),
                        stop=(kb == KB - 1),
                    )
                    # All three heads' row sums accumulate into one PSUM bank
                    # (rows 0/32/64). The bank is DVE-memset to zero up front,
                    # so plain accumulation (never start=True) is correct on
                    # both hardware and sim regardless of has_written state.
                    nc.tensor.matmul(
                        pdall[32 * h:32 * h + 1, :],
                        cast(ones_col[:]),
                        cast(est[:]),
                        start=False,
                        stop=(kb == KB - 1 and h == HPC - 1),
                        skip_group_check=True,
                    )
            for h in range(HPC):
                den_sb = smpool.tile([1, CHUNK], MMDT, tag=f"den{h}", name=f"den{h}")
                nc.scalar.copy(den_sb[:], pdall[32 * h:32 * h + 1, :])
                pb = ps_mm.tile([P, CHUNK], F32, tag="mm1", name="mm1")
                nc.tensor.matmul(
                    pb[:], cast(ones_row[:]), cast(den_sb[:]), start=True, stop=True
                )
                recip = smpool.tile([P, CHUNK], F32, tag=f"recip{h}", name=f"recip{h}")
                nc.vector.reciprocal(recip[:], pb[:])
                nc.vector.tensor_mul(oTn[h][:], po[h][:], recip[:])
            outproj_chunk(c, oTn)

        def attn_chunk(c, qT):
            # ---- attention for this q-chunk ----
            KB = 4 * c + 5 if c < NCHUNK - 1 else LT
            oTn = [opool.tile([P, CHUNK], MMDT, tag=f"oT{h}", name=f"oT{h}")
                   for h in range(HPC)]
            for h in range(HPC):
                po = ps_acc.tile([P, CHUNK], F32, tag="pvacc", name="pvacc", bufs=2)
                esum = None
                if denom_mode == "esum":
                    # accumulate exp tiles elementwise on DVE; a single
                    # ones-matmul at the end replicates the row sums to all
                    # partitions (replaces one PE stream per kb block)
                    esum = espool.tile([P, CHUNK], MMDT, tag="esum", name="esum")
                    pd = ps_aux.tile([P, CHUNK], F32, tag="aux", name="aux")
                else:
                    pd = ps_aux.tile([P if fused_denb else 1, CHUNK], F32,
                                     tag="aux", name="aux")
                for kb in range(KB):
                    # Diagonal-band blocks (j >= 1): every column below
                    # 128j-1 is fully masked, so restrict all ops to the
                    # valid column range (8B-aligned start). The skipped
                    # region of est is stale but never read.
                    j = kb - 4 * c
                    s0 = 128 * j - 2 if j >= 1 else 0
                    sl = slice(s0, CHUNK)
                    pst = ps_mm.tile([P, CHUNK], F32, tag="mm1", name="mm1")
                    nc.tensor.matmul(
                        pst[:, sl],
                        cast(kT[h][:, kb * P:(kb + 1) * P]),
                        cast(qT[h][:, sl]),
                        start=True,
                        stop=True,
                    )
                    est = estpool.tile([P, CHUNK], MMDT, tag="est", name="est")
                    nc.scalar.activation(est[:, sl], pst[:, sl], AF.Exp, scale=SCALE)
                    if j >= 0:
                        eng = nc.gpsimd if mask_gpsimd else nc.vector
                        eng.tensor_mul(est[:, sl], est[:, sl], masks[j][:, sl])
                    nc.tensor.matmul(
                        po[:, sl],
                        cast(vt[kb][:, h * P:(h + 1) * P]),
                        cast(est[:, sl]),
                        start=(kb == 0),
                        stop=(kb == KB - 1),
                    )
                    if denom_mode == "esum":
                        if kb == 0:
                            nc.vector.tensor_copy(esum[:, sl], est[:, sl])
                        else:
                            nc.vector.tensor_add(
                                esum[:, sl], esum[:, sl], est[:, sl]
                            )
                    else:
                        # row-sum accumulation; fused_denb replicates the sum
                        # to all 128 partitions (ones matrix) so no broadcast
                        # matmul is needed afterwards
                        nc.tensor.matmul(
                            pd[:, sl],
                            cast(ones_mat[:] if fused_denb else ones_col[:]),
                            cast(est[:, sl]),
                            start=(kb == 0),
                            stop=(kb == KB - 1),
                        )
                # normalize: oTn = po * (1 / rowsum) broadcast over partitions
                recip = smpool.tile([P, CHUNK], F32, tag="recip", name="recip")
                if denom_mode == "esum":
                    nc.tensor.matmul(
                        pd[:], cast(ones_mat[:]), cast(esum[:]),
                        start=True, stop=True,
                    )
                    nc.vector.reciprocal(recip[:], pd[:])
                elif fused_denb:
                    nc.vector.reciprocal(recip[:], pd[:])
                else:
                    den_sb = smpool.tile([1, CHUNK], MMDT, tag="den", name="den")
                    if den_on_dve:
                        nc.vector.tensor_copy(den_sb[:], pd[:])
                    else:
                        nc.scalar.copy(den_sb[:], pd[:])
                    if pb_in_mm1:
                        pb = ps_mm.tile([P, CHUNK], F32, tag="mm1", name="mm1")
                    else:
                        pb = ps_aux.tile([P, CHUNK], F32, tag="aux", name="aux")
                    nc.tensor.matmul(
                        pb[:], cast(ones_row[:]), cast(den_sb[:]),
                        start=True, stop=True,
                    )
                    nc.vector.reciprocal(recip[:], pb[:])
                nc.vector.tensor_mul(oTn[h][:], po[:], recip[:])
            outproj_chunk(c, oTn)

        def outproj_chunk(c, oTn):
            # ---- output projection for this chunk ----
            for do in range(DT):
                py = ps_acc.tile([P, CHUNK], F32, tag="acc", name="acc")
                for h in range(HPC):
                    nc.tensor.matmul(
                        py[:],
                        cast(wo[h][:, do * P:(do + 1) * P]),
                        cast(oTn[h][:]),
                        start=(h == 0),
                        stop=(h == HPC - 1),
                    )
                yst = ypool.tile([P, CHUNK], F32, tag="yst", name="yst")
                if yst_on_act:
                    nc.scalar.copy(yst[:], py[:])
                else:
                    nc.vector.tensor_copy(yst[:], py[:])
                nc.sync.dma_start(
                    out=y_d[do * P:(do + 1) * P, c * CHUNK:(c + 1) * CHUNK],
                    in_=yst[:],
                )

        # Pipeline: attention of chunk c needs K/V through block 4c+4, which
        # lives in chunk c+1's rows (the tril(k=1) one-token lookahead). So
        # run projections one chunk ahead of attention. reps>1 repeats the
        # whole compute for benchmarking (amortizes dispatch overhead).
        attn = attn_chunk_ileave if attn_mode == "ileave" else attn_chunk
        for _rep in range(reps):
            qTs = {}
            qTs[0] = proj_chunk(0, xT=xT0 if _rep == 0 else None)
            if attn_order == "small_last":
                # attn(c) only needs proj(c+1); run the smallest chunk (0)
                # last so the un-overlapped kernel tail is as short as
                # possible. Needs qT(0) alive until the end (qpool bufs).
                qTs[1] = proj_chunk(1)
                qTs[2] = proj_chunk(2)
                attn(1, qTs.pop(1))
                qTs[3] = proj_chunk(3)
                attn(2, qTs.pop(2))
                attn(3, qTs.pop(3))
                attn(0, qTs.pop(0))
            else:
                for c in range(1, NCHUNK):
                    qTs[c] = proj_chunk(c)
                    attn(c - 1, qTs.pop(c - 1))
                attn(NCHUNK - 1, qTs.pop(NCHUNK - 1))

    nc.compile()
    return nc


def shard_inputs(x, Wq, bq, Wk, bk, Wv, bv, Wo, bo):
    x = np.asarray(x, dtype=np.float32)
    in_maps = []
    for core in range(NCORES):
        b = core // 2
        g = core % 2
        sl = slice(g * DH, (g + 1) * DH)
        in_maps.append({
            "x": np.ascontiguousarray(x[b].T),
            "wq": np.ascontiguousarray(np.asarray(Wq, np.float32)[:, sl]),
            "wk": np.ascontiguousarray(np.asarray(Wk, np.float32)[:, sl]),
            "wv": np.ascontiguousarray(np.asarray(Wv, np.float32)[:, sl]),
            "wo": np.ascontiguousarray(np.asarray(Wo, np.float32)[sl, :]),
            "bq": np.ascontiguousarray(np.asarray(bq, np.float32)[sl]),
            "bk": np.ascontiguousarray(np.asarray(bk, np.float32)[sl]),
        })
    return in_maps


def unshard_output(results, Wo, bv, bo):
    out = np.empty((B, L, D), dtype=np.float32)
    for b in range(B):
        acc = results[2 * b]["y"] + results[2 * b + 1]["y"]  # [D, L]
        out[b] = acc.T
    corr = np.asarray(bo, np.float32) + np.asarray(bv, np.float32) @ np.asarray(
        Wo, np.float32
    )
    out += corr
    return out


def run(inputs, trace=False, **kw):
    if "nc" not in _cache:
        _cache["nc"] = build_nc()
    nc = _cache["nc"]
    in_maps = shard_inputs(**inputs)
    res = run_bass_kernel_spmd(nc, in_maps, list(range(NCORES)), trace=trace, **kw)
    out = unshard_output(res.results, inputs["Wo"], inputs["bv"], inputs["bo"])
    return out, res


def kernel(**inputs):
    out, _ = run(inputs)
    return out



# revision 23
# speedup vs baseline: 1.1161x; 1.1161x over previous
"""Causal self-attention TRN2 kernel.

Problem: B=4, L=2048, D=768, H=6 heads, head_dim=128, fp32, causal mask
tril(k=1) (each query row q attends to keys k <= q+1).

Sharding: 8 cores = 4 batches x 2 head-groups (3 heads each).
Each core computes, for its batch b and heads [3g, 3g+3):
    Q = x_b @ Wq[:, cols] + bq[cols]   (and K, V likewise)
    per head: S^T = K @ Q^T (scaled), P = exp(S) masked, O = P@V / rowsum
    y_core = (O_heads @ Wo[rows, :])^T          -> [768, 2048] partial
Host: out[b] = (y[2b] + y[2b+1])^T + bo + bv @ Wo   (attn rows sum to 1,
so the V bias contributes exactly bv @ Wo_rows to every output row).

Layout trick: everything is kept transposed (feature dim on partitions) so
every matmul has a 512-wide moving operand and can run at full PE rate in
float32r (x itself arrives host-transposed, so no on-chip transposes at
all). Softmax runs without max-subtraction (logits are O(1) here), with
row sums computed by a ones-vector matmul in the same transposed layout,
then broadcast via a rank-1 matmul for the normalization multiply.
Projections run one 512-column chunk ahead of attention because the
tril(k=1) mask lets each query attend one token into the future.
Diagonal-band blocks restrict S/exp/mask/PV/rowsum to the valid column
range (everything below 128j-2 is structurally masked), cutting ~12% of
PE work.
"""

import math
from contextlib import ExitStack

import numpy as np

import concourse.tile as tile
from concourse import bacc, mybir
from concourse.bass_utils import run_bass_kernel_spmd

F32 = mybir.dt.float32
F32R = mybir.dt.float32r
AF = mybir.ActivationFunctionType

B, L, D, H = 4, 2048, 768, 6
HD = 128           # head dim
HPC = 3            # heads per core
DH = HPC * HD      # 384: per-core projection width
NCORES = 8
P = 128
CHUNK = 512        # q-chunk width (moving-operand size)
NCHUNK = L // CHUNK
LT = L // P        # 16 L-tiles
DT = D // P        # 6 d-tiles
SCALE = 1.0 / math.sqrt(HD)

_cache = {}


def build_nc(mm_fast=True, enable_asserts=False, reps=1,
             mm1_bufs=3, aux_bufs=1, pb_in_mm1=False, den_on_dve=False,
             est_bufs=6, attn_mode="seq", mask_gpsimd=False, vt_on_act=False,
             fused_denb=True, qk_on_dve=False, yst_on_act=False,
             attn_order="pipeline", qt_bufs=2, denom_mode="mm",
             mask_mode="select", pv_bufs=2, esum_ways=1, esum_eng="vvv",
             xt_bufs=2, kv_parity=False):
    nc = bacc.Bacc(
        "TRN2",
        target_bir_lowering=False,
        debug=False,
        enable_asserts=enable_asserts,
        num_devices=NCORES,
    )
    x_d = nc.dram_tensor("x", [D, L], F32, kind="ExternalInput").ap()
    wq_d = nc.dram_tensor("wq", [D, DH], F32, kind="ExternalInput").ap()
    wk_d = nc.dram_tensor("wk", [D, DH], F32, kind="ExternalInput").ap()
    wv_d = nc.dram_tensor("wv", [D, DH], F32, kind="ExternalInput").ap()
    wo_d = nc.dram_tensor("wo", [DH, D], F32, kind="ExternalInput").ap()
    bq_d = nc.dram_tensor("bq", [DH], F32, kind="ExternalInput").ap()
    bk_d = nc.dram_tensor("bk", [DH], F32, kind="ExternalInput").ap()
    y_d = nc.dram_tensor("y", [D, L], F32, kind="ExternalOutput").ap()

    MMDT = F32R if mm_fast else F32
    cast = lambda ap: ap  # noqa: E731

    with tile.TileContext(nc) as tc, ExitStack() as ctx:
        const = ctx.enter_context(tc.tile_pool(name="const", bufs=1))
        wpool = ctx.enter_context(tc.tile_pool(name="wts", bufs=1))
        kvpool = ctx.enter_context(tc.tile_pool(name="kv", bufs=1))
        xtpool = ctx.enter_context(tc.tile_pool(name="xt", bufs=xt_bufs))
        qpool = ctx.enter_context(tc.tile_pool(name="qt", bufs=qt_bufs))
        estpool = ctx.enter_context(tc.tile_pool(name="est", bufs=est_bufs))
        smpool = ctx.enter_context(tc.tile_pool(name="sm", bufs=2))
        opool = ctx.enter_context(tc.tile_pool(name="ot", bufs=2))
        espool = ctx.enter_context(tc.tile_pool(name="esum", bufs=2))
        ypool = ctx.enter_context(tc.tile_pool(name="yst", bufs=3))
        ps_mm = ctx.enter_context(tc.tile_pool(name="psmm", bufs=mm1_bufs, space="PSUM"))
        ps_acc = ctx.enter_context(tc.tile_pool(name="psacc", bufs=2, space="PSUM"))
        if attn_mode != "pair":
            ps_aux = ctx.enter_context(
                tc.tile_pool(name="psaux", bufs=aux_bufs, space="PSUM"))

        ones_f32 = const.tile([P, 1], F32, tag="ones_f32", name="ones_f32")
        nc.vector.memset(ones_f32[:], 1.0)
        ones_col = const.tile([P, 1], MMDT, tag="ones_col", name="ones_col")
        nc.scalar.copy(ones_col[:], ones_f32[:])
        ones_mf = const.tile([P, P], F32, tag="ones_mf", name="ones_mf")
        nc.vector.memset(ones_mf[:], 1.0)
        ones_mat = const.tile([P, P], MMDT, tag="ones_mat", name="ones_mat")
        nc.scalar.copy(ones_mat[:], ones_mf[:])
        ones_rf = const.tile([1, P], F32, tag="ones_rf", name="ones_rf")
        nc.vector.memset(ones_rf[:], 1.0)
        ones_row = const.tile([1, P], MMDT, tag="ones_row", name="ones_row")
        nc.scalar.copy(ones_row[:], ones_rf[:])
        # 5 diagonal-band masks (0/1), shared by all chunks/heads.
        # mask[j][kp, qq] = 1 iff kp - qq <= 1 - 128*j
        masks = []
        if mask_mode == "mul":
            for j in range(5):
                mj = const.tile([P, CHUNK], F32, tag=f"mask{j}", name=f"mask{j}")
                nc.gpsimd.memset(mj[:], 1.0)
                # keep (mask=1) where kp - qq <= 1 - 128j, i.e. qq - kp + (1-128j) >= 0
                nc.gpsimd.affine_select(
                    out=mj[:],
                    in_=mj[:],
                    pattern=[[1, CHUNK]],
                    compare_op=mybir.AluOpType.is_ge,
                    fill=0.0,
                    base=1 - 128 * j,
                    channel_multiplier=-1,
                )
                masks.append(mj)

        def apply_mask(est, j, sl):
            # zero est where kp - qq > 1 - 128j. Only columns in [m0, m1) can
            # be invalid (the diagonal triangle); outside that, all kp valid.
            if mask_mode == "mul":
                eng = nc.gpsimd if mask_gpsimd else nc.vector
                eng.tensor_mul(est[:, sl], est[:, sl], masks[j][:, sl])
                return
            # select mode: in-place gpsimd affine_select on the narrow strip.
            # j=3: the matmul slice is widened to 256 (f32r quarter-rate
            # below 256-wide moving operands) but exp still starts at 382, so
            # the select also covers (and zero-fills) the never-written
            # [256, 382) junk region.
            if j == 0:
                m0, m1 = 0, 128
            elif j == 3:
                m0, m1 = 256, CHUNK
            else:
                m0 = 128 * j - 2
                m1 = min(128 * j + 126, CHUNK)
            nc.gpsimd.affine_select(
                out=est[:, m0:m1],
                in_=est[:, m0:m1],
                pattern=[[1, m1 - m0]],
                compare_op=mybir.AluOpType.is_ge,
                fill=0.0,
                base=m0 + 1 - 128 * j,
                channel_multiplier=-1,
            )

        x_3d = x_d.rearrange("(d p) l -> p d l", p=P)
        y_3d = y_d.rearrange("(d p) l -> p d l", p=P)

        def make_xT(c):
            # xT columns: block d lives at [d*CHUNK, (d+1)*CHUNK).
            # One batched DMA per chunk (6 separate DMAs would eat ~13us of
            # SP-sequencer time per chunk; batched is ~1.8us).
            xT = xtpool.tile([P, DT * CHUNK], MMDT, tag="xT", name="xT")
            nc.sync.dma_start(
                out=xT[:].rearrange("p (d c) -> p d c", c=CHUNK),
                in_=x_3d[:, :, c * CHUNK:(c + 1) * CHUNK].bitcast(MMDT),
            )
            return xT

        # DMA issue order: wq tiles and chunk-0 x columns first so the first
        # projection matmuls can start early; wk/wv next; wo/biases later.
        wq = []
        for d in range(DT):
            wq_t = wpool.tile([P, DH], MMDT, tag=f"wq{d}", name=f"wq{d}")
            nc.sync.dma_start(out=wq_t[:], in_=wq_d[d * P:(d + 1) * P, :].bitcast(MMDT))
            wq.append(wq_t)
        xT0 = make_xT(0)
        wk = []
        wv = []
        for d in range(DT):
            wk_t = wpool.tile([P, DH], MMDT, tag=f"wk{d}", name=f"wk{d}")
            nc.sync.dma_start(out=wk_t[:], in_=wk_d[d * P:(d + 1) * P, :].bitcast(MMDT))
            wk.append(wk_t)
            wv_t = wpool.tile([P, DH], MMDT, tag=f"wv{d}", name=f"wv{d}")
            nc.sync.dma_start(out=wv_t[:], in_=wv_d[d * P:(d + 1) * P, :].bitcast(MMDT))
            wv.append(wv_t)
        bq_t = []
        bk_t = []
        for h in range(HPC):
            bq_h = wpool.tile([P, 1], F32, tag=f"bq{h}", name=f"bq{h}")
            nc.sync.dma_start(
                out=bq_h[:], in_=bq_d[h * P:(h + 1) * P].rearrange("(p o) -> p o", o=1)
            )
            bq_t.append(bq_h)
            bk_h = wpool.tile([P, 1], F32, tag=f"bk{h}", name=f"bk{h}")
            nc.sync.dma_start(
                out=bk_h[:], in_=bk_d[h * P:(h + 1) * P].rearrange("(p o) -> p o", o=1)
            )
            bk_t.append(bk_h)
        wo = []
        for h in range(HPC):
            wo_t = wpool.tile([P, D], MMDT, tag=f"wo{h}", name=f"wo{h}")
            nc.sync.dma_start(out=wo_t[:], in_=wo_d[h * P:(h + 1) * P, :].bitcast(MMDT))
            wo.append(wo_t)

        # K^T per head [hd=128, L]; V per L-tile [kpos=128, 3*hd].
        # Two parity sets so rep r+1's K/V projection writes don't serialize
        # behind rep r's tail-chunk attention reads (kv_parity pingpong).
        nkv = 2 if kv_parity and reps > 1 else 1
        kT_sets = [[kvpool.tile([P, L], MMDT, tag=f"kT{h}_{s}", name=f"kT{h}_{s}")
                    for h in range(HPC)] for s in range(nkv)]
        vt_sets = [[kvpool.tile([P, DH], MMDT, tag=f"v{t}_{s}", name=f"v{t}_{s}")
                    for t in range(LT)] for s in range(nkv)]
        kT = kT_sets[0]
        vt = vt_sets[0]

        def proj_chunk(c, xT=None):
            # ---- x^T columns for this chunk (x arrives host-transposed) ----
            if xT is None:
                xT = make_xT(c)

            # ---- Q^T, K^T projections for this chunk ----
            qT = [qpool.tile([P, CHUNK], MMDT, tag=f"qT{h}", name=f"qT{h}")
                  for h in range(HPC)]
            for h in range(HPC):
                pq = ps_acc.tile([P, CHUNK], F32, tag="acc", name="acc")
                for d in range(DT):
                    nc.tensor.matmul(
                        pq[:],
                        cast(wq[d][:, h * P:(h + 1) * P]),
                        cast(xT[:, d * CHUNK:(d + 1) * CHUNK]),
                        start=(d == 0),
                        stop=(d == DT - 1),
                    )
                if qk_on_dve:
                    nc.vector.tensor_scalar_add(qT[h][:], pq[:], bq_t[h][:])
                else:
                    nc.scalar.activation(qT[h][:], pq[:], AF.Identity,
                                         bias=bq_t[h][:])
                pk = ps_acc.tile([P, CHUNK], F32, tag="acc", name="acc")
                for d in range(DT):
                    nc.tensor.matmul(
                        pk[:],
                        cast(wk[d][:, h * P:(h + 1) * P]),
                        cast(xT[:, d * CHUNK:(d + 1) * CHUNK]),
                        start=(d == 0),
                        stop=(d == DT - 1),
                    )
                if qk_on_dve:
                    nc.vector.tensor_scalar_add(
                        kT[h][:, c * CHUNK:(c + 1) * CHUNK], pk[:], bk_t[h][:]
                    )
                else:
                    nc.scalar.activation(
                        kT[h][:, c * CHUNK:(c + 1) * CHUNK], pk[:], AF.Identity,
                        bias=bk_t[h][:],
                    )

            # ---- V projection (natural layout) ----
            for i in range(CHUNK // P):
                t = c * (CHUNK // P) + i
                pv = ps_acc.tile([P, DH], F32, tag="acc", name="acc")
                for d in range(DT):
                    nc.tensor.matmul(
                        pv[:],
                        cast(xT[:, d * CHUNK + i * P: d * CHUNK + (i + 1) * P]),
                        cast(wv[d][:]),
                        start=(d == 0),
                        stop=(d == DT - 1),
                    )
                if vt_on_act:
                    nc.scalar.copy(vt[t][:], pv[:])
                else:
                    nc.vector.tensor_copy(vt[t][:], pv[:])
            return qT

        def attn_chunk_ileave(c, qT):
            # ---- attention, 3 heads interleaved per kb block ----
            # Denominators for all heads pack into one PSUM bank (rows
            # 0/32/64 -- tile_position requires 32-aligned output rows).
            # A single start=True (h0,kb0) clears the bank; the other heads'
            # first writes overwrite via the has_written bits.
            KB = 4 * c + 5 if c < NCHUNK - 1 else LT
            oTn = [opool.tile([P, CHUNK], MMDT, tag=f"oT{h}", name=f"oT{h}")
                   for h in range(HPC)]
            po = [ps_acc.tile([P, CHUNK], F32, tag=f"pv{h}", name=f"pv{h}", bufs=1)
                  for h in range(HPC)]
            pdall = ps_aux.tile([P, CHUNK], F32, tag="aux", name="aux", bufs=1)
            nc.vector.memset(pdall[:], 0.0)
            for kb in range(KB):
                j = kb - 4 * c
                for h in range(HPC):
                    pst = ps_mm.tile([P, CHUNK], F32, tag="mm1", name="mm1")
                    nc.tensor.matmul(
                        pst[:],
                        cast(kT[h][:, kb * P:(kb + 1) * P]),
                        cast(qT[h][:]),
                        start=True,
                        stop=True,
                    )
                    est = estpool.tile([P, CHUNK], MMDT, tag="est", name="est")
                    nc.scalar.activation(est[:], pst[:], AF.Exp, scale=SCALE)
                    if j >= 0:
                        apply_mask(est, j, slice(0, CHUNK))
                    nc.tensor.matmul(
                        po[h][:],
                        cast(vt[kb][:, h * P:(h + 1) * P]),
                        cast(est[:]),
                        start=(kb == 0),
                        stop=(kb == KB - 1),
                    )
                    # All three heads' row sums accumulate into one PSUM bank
                    # (rows 0/32/64). The bank is DVE-memset to zero up front,
                    # so plain accumulation (never start=True) is correct on
                    # both hardware and sim regardless of has_written state.
                    nc.tensor.matmul(
                        pdall[32 * h:32 * h + 1, :],
                        cast(ones_col[:]),
                        cast(est[:]),
                        start=False,
                        stop=(kb == KB - 1 and h == HPC - 1),
                        skip_group_check=True,
                    )
            for h in range(HPC):
                den_sb = smpool.tile([1, CHUNK], MMDT, tag=f"den{h}", name=f"den{h}")
                nc.scalar.copy(den_sb[:], pdall[32 * h:32 * h + 1, :])
                pb = ps_mm.tile([P, CHUNK], F32, tag="mm1", name="mm1")
                nc.tensor.matmul(
                    pb[:], cast(ones_row[:]), cast(den_sb[:]), start=True, stop=True
                )
                recip = smpool.tile([P, CHUNK], F32, tag=f"recip{h}", name=f"recip{h}")
                nc.vector.reciprocal(recip[:], pb[:])
                nc.vector.tensor_mul(oTn[h][:], po[h][:], recip[:])
            outproj_chunk(c, oTn)

        def attn_chunk(c, qT):
            # ---- attention for this q-chunk ----
            KB = 4 * c + 5 if c < NCHUNK - 1 else LT
            oTn = [opool.tile([P, CHUNK], MMDT, tag=f"oT{h}", name=f"oT{h}")
                   for h in range(HPC)]
            for h in range(HPC):
                po = ps_acc.tile([P, CHUNK], F32, tag="pvacc", name="pvacc",
                                 bufs=pv_bufs)
                esum = None
                if denom_mode == "esum":
                    # accumulate exp tiles elementwise (DVE or gpsimd per
                    # head, esum_ways-way tree to cut chain depth); the
                    # ones-matmuls at the end replicate the row sums to all
                    # partitions (replaces one PE stream per kb block)
                    eng = {"v": nc.vector, "g": nc.gpsimd}[esum_eng[h]]
                    esum = [espool.tile([P, CHUNK], MMDT, tag=f"esum{w}",
                                        name=f"esum{w}")
                            for w in range(esum_ways)]
                    esum_started = [False] * esum_ways
                    way_s0 = [0] * esum_ways
                    pd = ps_aux.tile([P, CHUNK], F32, tag="aux", name="aux")
                else:
                    pd = ps_aux.tile([P if fused_denb else 1, CHUNK], F32,
                                     tag="aux", name="aux")
                for kb in range(KB):
                    # Diagonal-band blocks (j >= 1): every column below
                    # 128j-1 is fully masked, so restrict all ops to the
                    # valid column range (8B-aligned start). The skipped
                    # region of est is stale but never read. In select mode
                    # the j=3 matmul slice is widened to 256 (f32r runs at
                    # quarter rate below 256-wide); the select zero-fills the
                    # never-exp'd [256, 382) region so PV/rowsum stay exact.
                    j = kb - 4 * c
                    s0 = 128 * j - 2 if j >= 1 else 0
                    e0 = s0
                    if mask_mode == "select" and j == 3:
                        s0 = 256
                    sl = slice(s0, CHUNK)
                    esl = slice(e0, CHUNK)
                    pst = ps_mm.tile([P, CHUNK], F32, tag="mm1", name="mm1")
                    nc.tensor.matmul(
                        pst[:, sl],
                        cast(kT[h][:, kb * P:(kb + 1) * P]),
                        cast(qT[h][:, sl]),
                        start=True,
                        stop=True,
                    )
                    est = estpool.tile([P, CHUNK], MMDT, tag="est", name="est")
                    nc.scalar.activation(est[:, esl], pst[:, esl], AF.Exp,
                                         scale=SCALE)
                    if j >= 0:
                        apply_mask(est, j, sl)
                    nc.tensor.matmul(
                        po[:, sl],
                        cast(vt[kb][:, h * P:(h + 1) * P]),
                        cast(est[:, sl]),
                        start=(kb == 0),
                        stop=(kb == KB - 1),
                    )
                    if denom_mode == "esum":
                        w = kb % esum_ways
                        if not esum_started[w]:
                            # first write per way covers the full remaining
                            # column range (sl ranges narrow as kb grows) so
                            # later adds always land on initialized data
                            nc.vector.tensor_copy(esum[w][:, sl], est[:, sl])
                            esum_started[w] = True
                            way_s0[w] = s0
                        else:
                            eng.tensor_add(
                                esum[w][:, sl], esum[w][:, sl], est[:, sl]
                            )
                    else:
                        # row-sum accumulation; fused_denb replicates the sum
                        # to all 128 partitions (ones matrix) so no broadcast
                        # matmul is needed afterwards
                        nc.tensor.matmul(
                            pd[:, sl],
                            cast(ones_mat[:] if fused_denb else ones_col[:]),
                            cast(est[:, sl]),
                            start=(kb == 0),
                            stop=(kb == KB - 1),
                        )
                # normalize: oTn = po * (1 / rowsum) broadcast over partitions
                recip = smpool.tile([P, CHUNK], F32, tag="recip", name="recip")
                if denom_mode == "esum":
                    nways = sum(esum_started)
                    wlast = nways - 1
                    for w in range(nways):
                        wsl = slice(way_s0[w], CHUNK)
                        nc.tensor.matmul(
                            pd[:, wsl], cast(ones_mat[:]), cast(esum[w][:, wsl]),
                            start=(w == 0), stop=(w == wlast),
                        )
                    nc.vector.reciprocal(recip[:], pd[:])
                elif fused_denb:
                    nc.vector.reciprocal(recip[:], pd[:])
                else:
                    den_sb = smpool.tile([1, CHUNK], MMDT, tag="den", name="den")
                    if den_on_dve:
                        nc.vector.tensor_copy(den_sb[:], pd[:])
                    else:
                        nc.scalar.copy(den_sb[:], pd[:])
                    if pb_in_mm1:
                        pb = ps_mm.tile([P, CHUNK], F32, tag="mm1", name="mm1")
                    else:
                        pb = ps_aux.tile([P, CHUNK], F32, tag="aux", name="aux")
                    nc.tensor.matmul(
                        pb[:], cast(ones_row[:]), cast(den_sb[:]),
                        start=True, stop=True,
                    )
                    nc.vector.reciprocal(recip[:], pb[:])
                nc.vector.tensor_mul(oTn[h][:], po[:], recip[:])
            outproj_chunk(c, oTn)

        def attn_chunk_pair(c, qT):
            # Paired kb blocks: one [P, 2*CHUNK] score-PSUM (2 banks) and one
            # exp per pair (halves ACT instruction count + sem traffic).
            # Masking via in-place gpsimd affine_select per diagonal half
            # (also zeroes the never-computed junk region of partial halves).
            # Denominators via esum accumulation on DVE/gpsimd (no PE rowsum
            # matmuls) + per-way ones matmuls at the end of each head.
            # The 2-wide lookahead block (j=4) rides in the last pair's
            # unused leading columns.
            KBP = 4 * c + 4 if c < NCHUNK - 1 else LT
            has_j4 = c < NCHUNK - 1
            npairs = KBP // 2
            kb4 = 4 * c + 4
            # S/PV matmul column starts per diagonal j (j=3 widened 382->256
            # to keep the f32r moving width >= 256), and affine-select spans.
            mm_s0 = {0: 0, 1: 126, 2: 254, 3: 256}
            sel_end = {0: 128, 1: 256, 2: 384, 3: 512}
            oTn = [opool.tile([P, CHUNK], MMDT, tag=f"oT{h}", name=f"oT{h}")
                   for h in range(HPC)]
            for h in range(HPC):
                po = ps_acc.tile([P, CHUNK], F32, tag="pvacc", name="pvacc",
                                 bufs=pv_bufs)
                eng = {"v": nc.vector, "g": nc.gpsimd}[esum_eng[h]]
                esum = [espool.tile([P, CHUNK], MMDT, tag=f"esum{w}",
                                    name=f"esum{w}") for w in range(esum_ways)]
                esum_started = [False] * esum_ways
                way_s0 = [0] * esum_ways

                def esum_add(kb, src, dst_sl, src_sl):
                    w = kb % esum_ways
                    if not esum_started[w]:
                        # first write per way has the widest range (ranges
                        # narrow as kb grows), so later adds land on
                        # initialized data
                        eng.tensor_copy(esum[w][:, dst_sl], src[:, src_sl])
                        esum_started[w] = True
                        way_s0[w] = dst_sl.start
                    else:
                        eng.tensor_add(esum[w][:, dst_sl], esum[w][:, dst_sl],
                                       src[:, src_sl])

                for p in range(npairs):
                    last_pair = p == npairs - 1
                    pair = ps_mm.tile([P, 2 * CHUNK], F32, tag="mm1", name="mm1")
                    est = estpool.tile([P, 2 * CHUNK], MMDT, tag="est", name="est")
                    js = []
                    for half in (0, 1):
                        kb = 2 * p + half
                        j = kb - 4 * c
                        s0 = mm_s0[j] if j >= 0 else 0
                        js.append((half, kb, j, s0))
                        nc.tensor.matmul(
                            pair[:, half * CHUNK + s0:(half + 1) * CHUNK],
                            cast(kT[h][:, kb * P:(kb + 1) * P]),
                            cast(qT[h][:, s0:CHUNK]),
                            start=True,
                            stop=True,
                        )
                    if has_j4 and last_pair:
                        nc.tensor.matmul(
                            pair[:, 0:2],
                            cast(kT[h][:, kb4 * P:(kb4 + 1) * P]),
                            cast(qT[h][:, CHUNK - 2:CHUNK]),
                            start=True,
                            stop=True,
                            skip_group_check=True,
                        )
                    x0 = js[0][3]
                    nc.scalar.activation(est[:, x0:2 * CHUNK],
                                         pair[:, x0:2 * CHUNK], AF.Exp,
                                         scale=SCALE)
                    for half, kb, j, s0 in js:
                        if j >= 0:
                            e0 = half * CHUNK
                            se = sel_end[j]
                            nc.gpsimd.affine_select(
                                out=est[:, e0:e0 + se],
                                in_=est[:, e0:e0 + se],
                                pattern=[[1, se]],
                                compare_op=mybir.AluOpType.is_ge,
                                fill=0.0,
                                base=1 - 128 * j,
                                channel_multiplier=-1,
                            )
                    if has_j4 and last_pair:
                        # j2's select zeroed est[:, 0:2]; overwrite with the
                        # lookahead exp, then mask it (only (kp=0, qq=511)
                        # survives)
                        nc.scalar.activation(est[:, 0:2], pair[:, 0:2],
                                             AF.Exp, scale=SCALE)
                        nc.gpsimd.affine_select(
                            out=est[:, 0:2],
                            in_=est[:, 0:2],
                            pattern=[[1, 2]],
                            compare_op=mybir.AluOpType.is_ge,
                            fill=0.0,
                            base=-1,
                            channel_multiplier=-1,
                        )
                    for half, kb, j, s0 in js:
                        sl = slice(s0, CHUNK)
                        nc.tensor.matmul(
                            po[:, sl],
                            cast(vt[kb][:, h * P:(h + 1) * P]),
                            cast(est[:, half * CHUNK + s0:(half + 1) * CHUNK]),
                            start=(p == 0 and half == 0),
                            stop=(last_pair and half == 1 and not has_j4),
                        )
                        esum_add(kb, est, sl,
                                 slice(half * CHUNK + s0, (half + 1) * CHUNK))
                    if has_j4 and last_pair:
                        nc.tensor.matmul(
                            po[:, CHUNK - 2:CHUNK],
                            cast(vt[kb4][:, h * P:(h + 1) * P]),
                            cast(est[:, 0:2]),
                            start=False,
                            stop=True,
                        )
                        esum_add(kb4, est, slice(CHUNK - 2, CHUNK), slice(0, 2))

                pd = ps_mm.tile([P, 2 * CHUNK], F32, tag="mm1", name="mm1")
                nways = sum(esum_started)
                for w in range(nways):
                    wsl = slice(way_s0[w], CHUNK)
                    nc.tensor.matmul(
                        pd[:, wsl], cast(ones_mat[:]), cast(esum[w][:, wsl]),
                        start=(w == 0), stop=(w == nways - 1),
                    )
                recip = smpool.tile([P, CHUNK], F32, tag="recip", name="recip")
                nc.vector.reciprocal(recip[:], pd[:, 0:CHUNK])
                nc.vector.tensor_mul(oTn[h][:], po[:], recip[:])
            outproj_chunk(c, oTn)

        def outproj_chunk(c, oTn):
            # ---- output projection for this chunk ----
            for do in range(DT):
                py = ps_acc.tile([P, CHUNK], F32, tag="acc", name="acc")
                for h in range(HPC):
                    nc.tensor.matmul(
                        py[:],
                        cast(wo[h][:, do * P:(do + 1) * P]),
                        cast(oTn[h][:]),
                        start=(h == 0),
                        stop=(h == HPC - 1),
                    )
                yst = ypool.tile([P, CHUNK], F32, tag="yst", name="yst")
                if yst_on_act:
                    nc.scalar.copy(yst[:], py[:])
                else:
                    nc.vector.tensor_copy(yst[:], py[:])
                nc.sync.dma_start(
                    out=y_d[do * P:(do + 1) * P, c * CHUNK:(c + 1) * CHUNK],
                    in_=yst[:],
                )

        # Pipeline: attention of chunk c needs K/V through block 4c+4, which
        # lives in chunk c+1's rows (the tril(k=1) one-token lookahead). So
        # run projections one chunk ahead of attention. reps>1 repeats the
        # whole compute for benchmarking (amortizes dispatch overhead).
        attn = {"ileave": attn_chunk_ileave, "pair": attn_chunk_pair,
                "seq": attn_chunk}[attn_mode]
        for _rep in range(reps):
            kT = kT_sets[_rep % nkv]
            vt = vt_sets[_rep % nkv]
            qTs = {}
            qTs[0] = proj_chunk(0, xT=xT0 if _rep == 0 else None)
            if attn_order == "small_last":
                # attn(c) only needs proj(c+1); run the smallest chunk (0)
                # last so the un-overlapped kernel tail is as short as
                # possible. Needs qT(0) alive until the end (qpool bufs).
                qTs[1] = proj_chunk(1)
                qTs[2] = proj_chunk(2)
                attn(1, qTs.pop(1))
                qTs[3] = proj_chunk(3)
                attn(2, qTs.pop(2))
                attn(3, qTs.pop(3))
                attn(0, qTs.pop(0))
            else:
                for c in range(1, NCHUNK):
                    qTs[c] = proj_chunk(c)
                    attn(c - 1, qTs.pop(c - 1))
                attn(NCHUNK - 1, qTs.pop(NCHUNK - 1))

    nc.compile()
    return nc


def shard_inputs(x, Wq, bq, Wk, bk, Wv, bv, Wo, bo):
    x = np.asarray(x, dtype=np.float32)
    in_maps = []
    for core in range(NCORES):
        b = core // 2
        g = core % 2
        sl = slice(g * DH, (g + 1) * DH)
        in_maps.append({
            "x": np.ascontiguousarray(x[b].T),
            "wq": np.ascontiguousarray(np.asarray(Wq, np.float32)[:, sl]),
            "wk": np.ascontiguousarray(np.asarray(Wk, np.float32)[:, sl]),
            "wv": np.ascontiguousarray(np.asarray(Wv, np.float32)[:, sl]),
            "wo": np.ascontiguousarray(np.asarray(Wo, np.float32)[sl, :]),
            "bq": np.ascontiguousarray(np.asarray(bq, np.float32)[sl]),
            "bk": np.ascontiguousarray(np.asarray(bk, np.float32)[sl]),
        })
    return in_maps


def unshard_output(results, Wo, bv, bo):
    out = np.empty((B, L, D), dtype=np.float32)
    for b in range(B):
        acc = results[2 * b]["y"] + results[2 * b + 1]["y"]  # [D, L]
        out[b] = acc.T
    corr = np.asarray(bo, np.float32) + np.asarray(bv, np.float32) @ np.asarray(
        Wo, np.float32
    )
    out += corr
    return out


def run(inputs, trace=False, **kw):
    if "nc" not in _cache:
        _cache["nc"] = build_nc()
    nc = _cache["nc"]
    in_maps = shard_inputs(**inputs)
    res = run_bass_kernel_spmd(nc, in_maps, list(range(NCORES)), trace=trace, **kw)
    out = unshard_output(res.results, inputs["Wo"], inputs["bv"], inputs["bo"])
    return out, res


def kernel(**inputs):
    out, _ = run(inputs)
    return out



# revision 34
# speedup vs baseline: 1.2199x; 1.0930x over previous
"""Causal self-attention TRN2 kernel.

Problem: B=4, L=2048, D=768, H=6 heads, head_dim=128, fp32, causal mask
tril(k=1) (each query row q attends to keys k <= q+1).

Sharding: 8 cores = 4 batches x 2 head-groups (3 heads each).
Each core computes, for its batch b and heads [3g, 3g+3):
    Q = x_b @ Wq[:, cols] + bq[cols]   (and K, V likewise)
    per head: S^T = K @ Q^T (scaled), P = exp(S) masked, O = P@V / rowsum
    y_core = (O_heads @ Wo[rows, :])^T          -> [768, 2048] partial
Host: out[b] = (y[2b] + y[2b+1])^T + bo + bv @ Wo   (attn rows sum to 1,
so the V bias contributes exactly bv @ Wo_rows to every output row).

Layout trick: everything is kept transposed (feature dim on partitions) so
every matmul has a 512-wide moving operand and can run at full PE rate in
float32r (x itself arrives host-transposed, so no on-chip transposes at
all). Softmax runs without max-subtraction (logits are O(1) here), with
row sums computed by an all-ones-matrix matmul in the same transposed
layout (fused_denb: the [128,128] ones stationary replicates the sum to
all partitions, so no broadcast matmul is needed), and the reciprocal via
the fast approximate DVE op (the exact InstReciprocal costs ~4us/tile on
HW and was worth ~48us of wall clock).
Projections run one 512-column chunk ahead of attention because the
tril(k=1) mask lets each query attend one token into the future.
Diagonal-band blocks restrict S/exp/mask/PV/rowsum to the valid column
range (everything below 128j-2 is structurally masked, with the j=3 slice
widened to 256 because f32r matmuls below 256-wide run at quarter rate).
Masking is in-place gpsimd affine_select on ~128-wide strips (DVE mask
multiplies measured ~19us slower on HW). x/y/weight DMAs are batched into
few multi-d descriptors, split 2-ways so two DMA engines run in parallel.
"""

import math
from contextlib import ExitStack

import numpy as np

import concourse.tile as tile
from concourse import bacc, mybir
from concourse.bass_utils import run_bass_kernel_spmd

F32 = mybir.dt.float32
F32R = mybir.dt.float32r
AF = mybir.ActivationFunctionType

B, L, D, H = 4, 2048, 768, 6
HD = 128           # head dim
HPC = 3            # heads per core
DH = HPC * HD      # 384: per-core projection width
NCORES = 8
P = 128
CHUNK = 512        # q-chunk width (moving-operand size)
NCHUNK = L // CHUNK
LT = L // P        # 16 L-tiles
DT = D // P        # 6 d-tiles
SCALE = 1.0 / math.sqrt(HD)

_cache = {}


def build_nc(mm_fast=True, enable_asserts=False, reps=1,
             mm1_bufs=3, aux_bufs=1, pb_in_mm1=False, den_on_dve=False,
             est_bufs=6, attn_mode="seq", mask_gpsimd=False, vt_on_act=False,
             fused_denb=True, qk_on_dve=False, yst_eng="v",
             attn_order="pipeline", qt_bufs=2, denom_mode="mm",
             mask_mode="select", pv_bufs=2, esum_ways=1, esum_eng="vvv",
             xt_bufs=2, kv_parity=False, recip_fast=True, xt_split=2,
             y_split=2):
    nc = bacc.Bacc(
        "TRN2",
        target_bir_lowering=False,
        debug=False,
        enable_asserts=enable_asserts,
        num_devices=NCORES,
    )
    x_d = nc.dram_tensor("x", [D, L], F32, kind="ExternalInput").ap()
    wq_d = nc.dram_tensor("wq", [D, DH], F32, kind="ExternalInput").ap()
    wk_d = nc.dram_tensor("wk", [D, DH], F32, kind="ExternalInput").ap()
    wv_d = nc.dram_tensor("wv", [D, DH], F32, kind="ExternalInput").ap()
    wo_d = nc.dram_tensor("wo", [DH, D], F32, kind="ExternalInput").ap()
    bq_d = nc.dram_tensor("bq", [DH], F32, kind="ExternalInput").ap()
    bk_d = nc.dram_tensor("bk", [DH], F32, kind="ExternalInput").ap()
    y_d = nc.dram_tensor("y", [D, L], F32, kind="ExternalOutput").ap()

    MMDT = F32R if mm_fast else F32
    cast = lambda ap: ap  # noqa: E731

    with tile.TileContext(nc) as tc, ExitStack() as ctx:
        const = ctx.enter_context(tc.tile_pool(name="const", bufs=1))
        wpool = ctx.enter_context(tc.tile_pool(name="wts", bufs=1))
        kvpool = ctx.enter_context(tc.tile_pool(name="kv", bufs=1))
        xtpool = ctx.enter_context(tc.tile_pool(name="xt", bufs=xt_bufs))
        qpool = ctx.enter_context(tc.tile_pool(name="qt", bufs=qt_bufs))
        estpool = ctx.enter_context(tc.tile_pool(name="est", bufs=est_bufs))
        smpool = ctx.enter_context(tc.tile_pool(name="sm", bufs=2))
        opool = ctx.enter_context(tc.tile_pool(name="ot", bufs=2))
        espool = ctx.enter_context(tc.tile_pool(name="esum", bufs=2))
        ypool = ctx.enter_context(tc.tile_pool(name="yst", bufs=2))
        ps_mm = ctx.enter_context(tc.tile_pool(name="psmm", bufs=mm1_bufs, space="PSUM"))
        ps_acc = ctx.enter_context(tc.tile_pool(name="psacc", bufs=2, space="PSUM"))
        if attn_mode != "pair":
            ps_aux = ctx.enter_context(
                tc.tile_pool(name="psaux", bufs=aux_bufs, space="PSUM"))

        ones_f32 = const.tile([P, 1], F32, tag="ones_f32", name="ones_f32")
        nc.vector.memset(ones_f32[:], 1.0)
        ones_col = const.tile([P, 1], MMDT, tag="ones_col", name="ones_col")
        nc.scalar.copy(ones_col[:], ones_f32[:])
        ones_mf = const.tile([P, P], F32, tag="ones_mf", name="ones_mf")
        nc.vector.memset(ones_mf[:], 1.0)
        ones_mat = const.tile([P, P], MMDT, tag="ones_mat", name="ones_mat")
        nc.scalar.copy(ones_mat[:], ones_mf[:])
        ones_rf = const.tile([1, P], F32, tag="ones_rf", name="ones_rf")
        nc.vector.memset(ones_rf[:], 1.0)
        ones_row = const.tile([1, P], MMDT, tag="ones_row", name="ones_row")
        nc.scalar.copy(ones_row[:], ones_rf[:])
        # 5 diagonal-band masks (0/1), shared by all chunks/heads.
        # mask[j][kp, qq] = 1 iff kp - qq <= 1 - 128*j
        masks = []
        if mask_mode == "mul":
            for j in range(5):
                mj = const.tile([P, CHUNK], F32, tag=f"mask{j}", name=f"mask{j}")
                nc.gpsimd.memset(mj[:], 1.0)
                # keep (mask=1) where kp - qq <= 1 - 128j, i.e. qq - kp + (1-128j) >= 0
                nc.gpsimd.affine_select(
                    out=mj[:],
                    in_=mj[:],
                    pattern=[[1, CHUNK]],
                    compare_op=mybir.AluOpType.is_ge,
                    fill=0.0,
                    base=1 - 128 * j,
                    channel_multiplier=-1,
                )
                masks.append(mj)

        def apply_mask(est, j, sl):
            # zero est where kp - qq > 1 - 128j. Only columns in [m0, m1) can
            # be invalid (the diagonal triangle); outside that, all kp valid.
            if mask_mode == "mul":
                eng = nc.gpsimd if mask_gpsimd else nc.vector
                eng.tensor_mul(est[:, sl], est[:, sl], masks[j][:, sl])
                return
            # select mode: in-place gpsimd affine_select on the narrow strip.
            # j=3: the matmul slice is widened to 256 (f32r quarter-rate
            # below 256-wide moving operands) but exp still starts at 382, so
            # the select also covers (and zero-fills) the never-written
            # [256, 382) junk region.
            if j == 0:
                m0, m1 = 0, 128
            elif j == 3:
                m0, m1 = 256, CHUNK
            else:
                m0 = 128 * j - 2
                m1 = min(128 * j + 126, CHUNK)
            nc.gpsimd.affine_select(
                out=est[:, m0:m1],
                in_=est[:, m0:m1],
                pattern=[[1, m1 - m0]],
                compare_op=mybir.AluOpType.is_ge,
                fill=0.0,
                base=m0 + 1 - 128 * j,
                channel_multiplier=-1,
            )

        x_3d = x_d.rearrange("(d p) l -> p d l", p=P)
        y_3d = y_d.rearrange("(d p) l -> p d l", p=P)

        # denominators are sums of exps in [~1e-3, ~1e4]: far from the
        # approx-reciprocal edge cases, and 18 correct bits is plenty for
        # softmax normalization.
        recip_fn = (nc.vector.reciprocal_approx_fast if recip_fast
                    else nc.vector.reciprocal)

        def make_xT(c):
            # xT columns: block d lives at [d*CHUNK, (d+1)*CHUNK).
            # xt_split>1 splits the chunk DMA so multiple DMA engines run in
            # parallel (one descriptor chain executes on a single engine).
            xT = xtpool.tile([P, DT * CHUNK], MMDT, tag="xT", name="xT")
            dper = DT // xt_split
            for i in range(xt_split):
                d0 = i * dper
                nc.sync.dma_start(
                    out=xT[:, d0 * CHUNK:(d0 + dper) * CHUNK].rearrange(
                        "p (d c) -> p d c", c=CHUNK),
                    in_=x_3d[:, d0:d0 + dper,
                             c * CHUNK:(c + 1) * CHUNK].bitcast(MMDT),
                )
            return xT

        # DMA issue order: wq and chunk-0 x columns first so the first
        # projection matmuls can start early; wk/wv next; wo/biases later.
        # One batched DMA per weight tensor (d-tiles side by side).
        def load_w(w3, dram, dt_, width):
            nc.sync.dma_start(
                out=w3[:].rearrange("p (d m) -> p d m", m=width),
                in_=dram.rearrange("(d p) m -> p d m", p=P).bitcast(dt_),
            )

        wq_all = wpool.tile([P, DT * DH], MMDT, tag="wq", name="wq")
        load_w(wq_all, wq_d, MMDT, DH)

        xT0 = make_xT(0)
        wk_all = wpool.tile([P, DT * DH], MMDT, tag="wk", name="wk")
        load_w(wk_all, wk_d, MMDT, DH)

        wv_all = wpool.tile([P, DT * DH], MMDT, tag="wv", name="wv")
        load_w(wv_all, wv_d, MMDT, DH)

        bq_all = wpool.tile([P, HPC], F32, tag="bq", name="bq")
        nc.sync.dma_start(
            out=bq_all[:], in_=bq_d.rearrange("(h p) -> p h", p=P))
        bk_all = wpool.tile([P, HPC], F32, tag="bk", name="bk")
        nc.sync.dma_start(
            out=bk_all[:], in_=bk_d.rearrange("(h p) -> p h", p=P))

        wo_all = wpool.tile([P, HPC * D], MMDT, tag="wo", name="wo")
        load_w(wo_all, wo_d, MMDT, D)


        # K^T per head [hd=128, L]; V per L-tile [kpos=128, 3*hd].
        # Two parity sets so rep r+1's K/V projection writes don't serialize
        # behind rep r's tail-chunk attention reads (kv_parity pingpong).
        nkv = 2 if kv_parity and reps > 1 else 1
        kT_sets = [[kvpool.tile([P, L], MMDT, tag=f"kT{h}_{s}", name=f"kT{h}_{s}")
                    for h in range(HPC)] for s in range(nkv)]
        vt_sets = [[kvpool.tile([P, DH], MMDT, tag=f"v{t}_{s}", name=f"v{t}_{s}")
                    for t in range(LT)] for s in range(nkv)]
        kT = kT_sets[0]
        vt = vt_sets[0]

        def proj_chunk(c, xT=None):
            # ---- x^T columns for this chunk (x arrives host-transposed) ----
            if xT is None:
                xT = make_xT(c)

            # ---- Q^T, K^T projections for this chunk ----
            qT = [qpool.tile([P, CHUNK], MMDT, tag=f"qT{h}", name=f"qT{h}")
                  for h in range(HPC)]
            for h in range(HPC):
                pq = ps_acc.tile([P, CHUNK], F32, tag="acc", name="acc")
                for d in range(DT):
                    nc.tensor.matmul(
                        pq[:],
                        cast(wq_all[:, d * DH + h * P:d * DH + (h + 1) * P]),
                        cast(xT[:, d * CHUNK:(d + 1) * CHUNK]),
                        start=(d == 0),
                        stop=(d == DT - 1),
                    )
                if qk_on_dve:
                    nc.vector.tensor_scalar_add(qT[h][:], pq[:], bq_all[:, h:h + 1])
                else:
                    nc.scalar.activation(qT[h][:], pq[:], AF.Identity,
                                         bias=bq_all[:, h:h + 1])
                pk = ps_acc.tile([P, CHUNK], F32, tag="acc", name="acc")
                for d in range(DT):
                    nc.tensor.matmul(
                        pk[:],
                        cast(wk_all[:, d * DH + h * P:d * DH + (h + 1) * P]),
                        cast(xT[:, d * CHUNK:(d + 1) * CHUNK]),
                        start=(d == 0),
                        stop=(d == DT - 1),
                    )
                if qk_on_dve:
                    nc.vector.tensor_scalar_add(
                        kT[h][:, c * CHUNK:(c + 1) * CHUNK], pk[:], bk_all[:, h:h + 1]
                    )
                else:
                    nc.scalar.activation(
                        kT[h][:, c * CHUNK:(c + 1) * CHUNK], pk[:], AF.Identity,
                        bias=bk_all[:, h:h + 1],
                    )

            # ---- V projection (natural layout) ----
            for i in range(CHUNK // P):
                t = c * (CHUNK // P) + i
                pv = ps_acc.tile([P, DH], F32, tag="acc", name="acc")
                for d in range(DT):
                    nc.tensor.matmul(
                        pv[:],
                        cast(xT[:, d * CHUNK + i * P: d * CHUNK + (i + 1) * P]),
                        cast(wv_all[:, d * DH:(d + 1) * DH]),
                        start=(d == 0),
                        stop=(d == DT - 1),
                    )
                if vt_on_act:
                    nc.scalar.copy(vt[t][:], pv[:])
                else:
                    nc.vector.tensor_copy(vt[t][:], pv[:])
            return qT

        def attn_chunk_ileave(c, qT):
            # ---- attention, 3 heads interleaved per kb block ----
            # Denominators for all heads pack into one PSUM bank (rows
            # 0/32/64 -- tile_position requires 32-aligned output rows).
            # A single start=True (h0,kb0) clears the bank; the other heads'
            # first writes overwrite via the has_written bits.
            KB = 4 * c + 5 if c < NCHUNK - 1 else LT
            oTn = [opool.tile([P, CHUNK], MMDT, tag=f"oT{h}", name=f"oT{h}")
                   for h in range(HPC)]
            po = [ps_acc.tile([P, CHUNK], F32, tag=f"pv{h}", name=f"pv{h}", bufs=1)
                  for h in range(HPC)]
            pdall = ps_aux.tile([P, CHUNK], F32, tag="aux", name="aux", bufs=1)
            nc.vector.memset(pdall[:], 0.0)
            for kb in range(KB):
                j = kb - 4 * c
                for h in range(HPC):
                    pst = ps_mm.tile([P, CHUNK], F32, tag="mm1", name="mm1")
                    nc.tensor.matmul(
                        pst[:],
                        cast(kT[h][:, kb * P:(kb + 1) * P]),
                        cast(qT[h][:]),
                        start=True,
                        stop=True,
                    )
                    est = estpool.tile([P, CHUNK], MMDT, tag="est", name="est")
                    nc.scalar.activation(est[:], pst[:], AF.Exp, scale=SCALE)
                    if j >= 0:
                        apply_mask(est, j, slice(0, CHUNK))
                    nc.tensor.matmul(
                        po[h][:],
                        cast(vt[kb][:, h * P:(h + 1) * P]),
                        cast(est[:]),
                        start=(kb == 0),
                        stop=(kb == KB - 1),
                    )
                    # All three heads' row sums accumulate into one PSUM bank
                    # (rows 0/32/64). The bank is DVE-memset to zero up front,
                    # so plain accumulation (never start=True) is correct on
                    # both hardware and sim regardless of has_written state.
                    nc.tensor.matmul(
                        pdall[32 * h:32 * h + 1, :],
                        cast(ones_col[:]),
                        cast(est[:]),
                        start=False,
                        stop=(kb == KB - 1 and h == HPC - 1),
                        skip_group_check=True,
                    )
            for h in range(HPC):
                den_sb = smpool.tile([1, CHUNK], MMDT, tag=f"den{h}", name=f"den{h}")
                nc.scalar.copy(den_sb[:], pdall[32 * h:32 * h + 1, :])
                pb = ps_mm.tile([P, CHUNK], F32, tag="mm1", name="mm1")
                nc.tensor.matmul(
                    pb[:], cast(ones_row[:]), cast(den_sb[:]), start=True, stop=True
                )
                recip = smpool.tile([P, CHUNK], F32, tag=f"recip{h}", name=f"recip{h}")
                nc.vector.reciprocal(recip[:], pb[:])
                nc.vector.tensor_mul(oTn[h][:], po[h][:], recip[:])
            outproj_chunk(c, oTn)

        def attn_chunk(c, qT):
            # ---- attention for this q-chunk ----
            KB = 4 * c + 5 if c < NCHUNK - 1 else LT
            oTn = [opool.tile([P, CHUNK], MMDT, tag=f"oT{h}", name=f"oT{h}")
                   for h in range(HPC)]
            for h in range(HPC):
                po = ps_acc.tile([P, CHUNK], F32, tag="pvacc", name="pvacc",
                                 bufs=pv_bufs)
                esum = None
                if denom_mode == "esum":
                    # accumulate exp tiles elementwise (DVE or gpsimd per
                    # head, esum_ways-way tree to cut chain depth); the
                    # ones-matmuls at the end replicate the row sums to all
                    # partitions (replaces one PE stream per kb block)
                    eng = {"v": nc.vector, "g": nc.gpsimd}[esum_eng[h]]
                    esum = [espool.tile([P, CHUNK], MMDT, tag=f"esum{w}",
                                        name=f"esum{w}")
                            for w in range(esum_ways)]
                    esum_started = [False] * esum_ways
                    way_s0 = [0] * esum_ways
                    pd = ps_aux.tile([P, CHUNK], F32, tag="aux", name="aux")
                else:
                    pd = ps_aux.tile([P if fused_denb else 1, CHUNK], F32,
                                     tag="aux", name="aux")
                for kb in range(KB):
                    # Diagonal-band blocks (j >= 1): every column below
                    # 128j-1 is fully masked, so restrict all ops to the
                    # valid column range (8B-aligned start). The skipped
                    # region of est is stale but never read. In select mode
                    # the j=3 matmul slice is widened to 256 (f32r runs at
                    # quarter rate below 256-wide); the select zero-fills the
                    # never-exp'd [256, 382) region so PV/rowsum stay exact.
                    j = kb - 4 * c
                    s0 = 128 * j - 2 if j >= 1 else 0
                    e0 = s0
                    if mask_mode == "select" and j == 3:
                        s0 = 256
                    sl = slice(s0, CHUNK)
                    esl = slice(e0, CHUNK)
                    pst = ps_mm.tile([P, CHUNK], F32, tag="mm1", name="mm1")
                    nc.tensor.matmul(
                        pst[:, sl],
                        cast(kT[h][:, kb * P:(kb + 1) * P]),
                        cast(qT[h][:, sl]),
                        start=True,
                        stop=True,
                    )
                    est = estpool.tile([P, CHUNK], MMDT, tag="est", name="est")
                    nc.scalar.activation(est[:, esl], pst[:, esl], AF.Exp,
                                         scale=SCALE)
                    if j >= 0:
                        apply_mask(est, j, sl)
                    nc.tensor.matmul(
                        po[:, sl],
                        cast(vt[kb][:, h * P:(h + 1) * P]),
                        cast(est[:, sl]),
                        start=(kb == 0),
                        stop=(kb == KB - 1),
                    )
                    if denom_mode == "esum":
                        w = kb % esum_ways
                        if not esum_started[w]:
                            # first write per way covers the full remaining
                            # column range (sl ranges narrow as kb grows) so
                            # later adds always land on initialized data
                            nc.vector.tensor_copy(esum[w][:, sl], est[:, sl])
                            esum_started[w] = True
                            way_s0[w] = s0
                        else:
                            eng.tensor_add(
                                esum[w][:, sl], esum[w][:, sl], est[:, sl]
                            )
                    else:
                        # row-sum accumulation; fused_denb replicates the sum
                        # to all 128 partitions (ones matrix) so no broadcast
                        # matmul is needed afterwards
                        nc.tensor.matmul(
                            pd[:, sl],
                            cast(ones_mat[:] if fused_denb else ones_col[:]),
                            cast(est[:, sl]),
                            start=(kb == 0),
                            stop=(kb == KB - 1),
                        )
                # normalize: oTn = po * (1 / rowsum) broadcast over partitions
                recip = smpool.tile([P, CHUNK], F32, tag="recip", name="recip")
                if denom_mode == "esum":
                    nways = sum(esum_started)
                    wlast = nways - 1
                    for w in range(nways):
                        wsl = slice(way_s0[w], CHUNK)
                        nc.tensor.matmul(
                            pd[:, wsl], cast(ones_mat[:]), cast(esum[w][:, wsl]),
                            start=(w == 0), stop=(w == wlast),
                        )
                    recip_fn(recip[:], pd[:])
                elif fused_denb:
                    recip_fn(recip[:], pd[:])
                else:
                    den_sb = smpool.tile([1, CHUNK], MMDT, tag="den", name="den")
                    if den_on_dve:
                        nc.vector.tensor_copy(den_sb[:], pd[:])
                    else:
                        nc.scalar.copy(den_sb[:], pd[:])
                    if pb_in_mm1:
                        pb = ps_mm.tile([P, CHUNK], F32, tag="mm1", name="mm1")
                    else:
                        pb = ps_aux.tile([P, CHUNK], F32, tag="aux", name="aux")
                    nc.tensor.matmul(
                        pb[:], cast(ones_row[:]), cast(den_sb[:]),
                        start=True, stop=True,
                    )
                    nc.vector.reciprocal(recip[:], pb[:])
                nc.vector.tensor_mul(oTn[h][:], po[:], recip[:])
            outproj_chunk(c, oTn)

        def attn_chunk_pair(c, qT):
            # Paired kb blocks: one [P, 2*CHUNK] score-PSUM (2 banks) and one
            # exp per pair (halves ACT instruction count + sem traffic).
            # Masking via in-place gpsimd affine_select per diagonal half
            # (also zeroes the never-computed junk region of partial halves).
            # Denominators via esum accumulation on DVE/gpsimd (no PE rowsum
            # matmuls) + per-way ones matmuls at the end of each head.
            # The 2-wide lookahead block (j=4) rides in the last pair's
            # unused leading columns.
            KBP = 4 * c + 4 if c < NCHUNK - 1 else LT
            has_j4 = c < NCHUNK - 1
            npairs = KBP // 2
            kb4 = 4 * c + 4
            # S/PV matmul column starts per diagonal j (j=3 widened 382->256
            # to keep the f32r moving width >= 256), and affine-select spans.
            mm_s0 = {0: 0, 1: 126, 2: 254, 3: 256}
            sel_end = {0: 128, 1: 256, 2: 384, 3: 512}
            oTn = [opool.tile([P, CHUNK], MMDT, tag=f"oT{h}", name=f"oT{h}")
                   for h in range(HPC)]
            for h in range(HPC):
                po = ps_acc.tile([P, CHUNK], F32, tag="pvacc", name="pvacc",
                                 bufs=pv_bufs)
                eng = {"v": nc.vector, "g": nc.gpsimd}[esum_eng[h]]
                esum = [espool.tile([P, CHUNK], MMDT, tag=f"esum{w}",
                                    name=f"esum{w}") for w in range(esum_ways)]
                esum_started = [False] * esum_ways
                way_s0 = [0] * esum_ways

                def esum_add(kb, src, dst_sl, src_sl):
                    w = kb % esum_ways
                    if not esum_started[w]:
                        # first write per way has the widest range (ranges
                        # narrow as kb grows), so later adds land on
                        # initialized data
                        eng.tensor_copy(esum[w][:, dst_sl], src[:, src_sl])
                        esum_started[w] = True
                        way_s0[w] = dst_sl.start
                    else:
                        eng.tensor_add(esum[w][:, dst_sl], esum[w][:, dst_sl],
                                       src[:, src_sl])

                for p in range(npairs):
                    last_pair = p == npairs - 1
                    pair = ps_mm.tile([P, 2 * CHUNK], F32, tag="mm1", name="mm1")
                    est = estpool.tile([P, 2 * CHUNK], MMDT, tag="est", name="est")
                    js = []
                    for half in (0, 1):
                        kb = 2 * p + half
                        j = kb - 4 * c
                        s0 = mm_s0[j] if j >= 0 else 0
                        js.append((half, kb, j, s0))
                        nc.tensor.matmul(
                            pair[:, half * CHUNK + s0:(half + 1) * CHUNK],
                            cast(kT[h][:, kb * P:(kb + 1) * P]),
                            cast(qT[h][:, s0:CHUNK]),
                            start=True,
                            stop=True,
                        )
                    if has_j4 and last_pair:
                        nc.tensor.matmul(
                            pair[:, 0:2],
                            cast(kT[h][:, kb4 * P:(kb4 + 1) * P]),
                            cast(qT[h][:, CHUNK - 2:CHUNK]),
                            start=True,
                            stop=True,
                            skip_group_check=True,
                        )
                    x0 = js[0][3]
                    nc.scalar.activation(est[:, x0:2 * CHUNK],
                                         pair[:, x0:2 * CHUNK], AF.Exp,
                                         scale=SCALE)
                    for half, kb, j, s0 in js:
                        if j >= 0:
                            e0 = half * CHUNK
                            se = sel_end[j]
                            nc.gpsimd.affine_select(
                                out=est[:, e0:e0 + se],
                                in_=est[:, e0:e0 + se],
                                pattern=[[1, se]],
                                compare_op=mybir.AluOpType.is_ge,
                                fill=0.0,
                                base=1 - 128 * j,
                                channel_multiplier=-1,
                            )
                    if has_j4 and last_pair:
                        # j2's select zeroed est[:, 0:2]; overwrite with the
                        # lookahead exp, then mask it (only (kp=0, qq=511)
                        # survives)
                        nc.scalar.activation(est[:, 0:2], pair[:, 0:2],
                                             AF.Exp, scale=SCALE)
                        nc.gpsimd.affine_select(
                            out=est[:, 0:2],
                            in_=est[:, 0:2],
                            pattern=[[1, 2]],
                            compare_op=mybir.AluOpType.is_ge,
                            fill=0.0,
                            base=-1,
                            channel_multiplier=-1,
                        )
                    for half, kb, j, s0 in js:
                        sl = slice(s0, CHUNK)
                        nc.tensor.matmul(
                            po[:, sl],
                            cast(vt[kb][:, h * P:(h + 1) * P]),
                            cast(est[:, half * CHUNK + s0:(half + 1) * CHUNK]),
                            start=(p == 0 and half == 0),
                            stop=(last_pair and half == 1 and not has_j4),
                        )
                        esum_add(kb, est, sl,
                                 slice(half * CHUNK + s0, (half + 1) * CHUNK))
                    if has_j4 and last_pair:
                        nc.tensor.matmul(
                            po[:, CHUNK - 2:CHUNK],
                            cast(vt[kb4][:, h * P:(h + 1) * P]),
                            cast(est[:, 0:2]),
                            start=False,
                            stop=True,
                        )
                        esum_add(kb4, est, slice(CHUNK - 2, CHUNK), slice(0, 2))

                pd = ps_mm.tile([P, 2 * CHUNK], F32, tag="mm1", name="mm1")
                nways = sum(esum_started)
                for w in range(nways):
                    wsl = slice(way_s0[w], CHUNK)
                    nc.tensor.matmul(
                        pd[:, wsl], cast(ones_mat[:]), cast(esum[w][:, wsl]),
                        start=(w == 0), stop=(w == nways - 1),
                    )
                recip = smpool.tile([P, CHUNK], F32, tag="recip", name="recip")
                nc.vector.reciprocal(recip[:], pd[:, 0:CHUNK])
                nc.vector.tensor_mul(oTn[h][:], po[:], recip[:])
            outproj_chunk(c, oTn)

        def outproj_chunk(c, oTn):
            # ---- output projection for this chunk (one batched y-DMA) ----
            ybig = ypool.tile([P, DT * CHUNK], F32, tag="yst", name="yst")
            for do in range(DT):
                py = ps_acc.tile([P, CHUNK], F32, tag="acc", name="acc")
                for h in range(HPC):
                    nc.tensor.matmul(
                        py[:],
                        cast(wo_all[:, h * D + do * P:h * D + (do + 1) * P]),
                        cast(oTn[h][:]),
                        start=(h == 0),
                        stop=(h == HPC - 1),
                    )
                ysl = ybig[:, do * CHUNK:(do + 1) * CHUNK]
                if yst_eng == "a":
                    nc.scalar.copy(ysl, py[:])
                elif yst_eng == "g":
                    nc.gpsimd.tensor_copy(ysl, py[:])
                else:
                    nc.vector.tensor_copy(ysl, py[:])
            dper = DT // y_split
            for i in range(y_split):
                d0 = i * dper
                nc.sync.dma_start(
                    out=y_3d[:, d0:d0 + dper, c * CHUNK:(c + 1) * CHUNK],
                    in_=ybig[:, d0 * CHUNK:(d0 + dper) * CHUNK].rearrange(
                        "p (d c) -> p d c", c=CHUNK),
                )

        # Pipeline: attention of chunk c needs K/V through block 4c+4, which
        # lives in chunk c+1's rows (the tril(k=1) one-token lookahead). So
        # run projections one chunk ahead of attention. reps>1 repeats the
        # whole compute for benchmarking (amortizes dispatch overhead).
        attn = {"ileave": attn_chunk_ileave, "pair": attn_chunk_pair,
                "seq": attn_chunk}[attn_mode]
        for _rep in range(reps):
            kT = kT_sets[_rep % nkv]
            vt = vt_sets[_rep % nkv]
            qTs = {}
            qTs[0] = proj_chunk(0, xT=xT0 if _rep == 0 else None)
            if attn_order == "small_last":
                # attn(c) only needs proj(c+1); run the smallest chunk (0)
                # last so the un-overlapped kernel tail is as short as
                # possible. Needs qT(0) alive until the end (qpool bufs).
                qTs[1] = proj_chunk(1)
                qTs[2] = proj_chunk(2)
                attn(1, qTs.pop(1))
                qTs[3] = proj_chunk(3)
                attn(2, qTs.pop(2))
                attn(3, qTs.pop(3))
                attn(0, qTs.pop(0))
            else:
                for c in range(1, NCHUNK):
                    qTs[c] = proj_chunk(c)
                    attn(c - 1, qTs.pop(c - 1))
                attn(NCHUNK - 1, qTs.pop(NCHUNK - 1))

    nc.compile()
    return nc


def shard_inputs(x, Wq, bq, Wk, bk, Wv, bv, Wo, bo):
    x = np.asarray(x, dtype=np.float32)
    in_maps = []
    for core in range(NCORES):
        b = core // 2
        g = core % 2
        sl = slice(g * DH, (g + 1) * DH)
        in_maps.append({
            "x": np.ascontiguousarray(x[b].T),
            "wq": np.ascontiguousarray(np.asarray(Wq, np.float32)[:, sl]),
            "wk": np.ascontiguousarray(np.asarray(Wk, np.float32)[:, sl]),
            "wv": np.ascontiguousarray(np.asarray(Wv, np.float32)[:, sl]),
            "wo": np.ascontiguousarray(np.asarray(Wo, np.float32)[sl, :]),
            "bq": np.ascontiguousarray(np.asarray(bq, np.float32)[sl]),
            "bk": np.ascontiguousarray(np.asarray(bk, np.float32)[sl]),
        })
    return in_maps


def unshard_output(results, Wo, bv, bo):
    out = np.empty((B, L, D), dtype=np.float32)
    for b in range(B):
        acc = results[2 * b]["y"] + results[2 * b + 1]["y"]  # [D, L]
        out[b] = acc.T
    corr = np.asarray(bo, np.float32) + np.asarray(bv, np.float32) @ np.asarray(
        Wo, np.float32
    )
    out += corr
    return out


def run(inputs, trace=False, **kw):
    if "nc" not in _cache:
        _cache["nc"] = build_nc()
    nc = _cache["nc"]
    in_maps = shard_inputs(**inputs)
    res = run_bass_kernel_spmd(nc, in_maps, list(range(NCORES)), trace=trace, **kw)
    out = unshard_output(res.results, inputs["Wo"], inputs["bv"], inputs["bo"])
    return out, res


def kernel(**inputs):
    out, _ = run(inputs)
    return out



# revision 37
# speedup vs baseline: 1.3172x; 1.0797x over previous
"""Causal self-attention TRN2 kernel.

Problem: B=4, L=2048, D=768, H=6 heads, head_dim=128, fp32, causal mask
tril(k=1) (each query row q attends to keys k <= q+1).

Sharding: 8 cores = 4 batches x 2 head-groups (3 heads each).
Each core computes, for its batch b and heads [3g, 3g+3):
    Q = x_b @ Wq[:, cols] + bq[cols]   (and K, V likewise)
    per head: S^T = K @ Q^T (scaled), P = exp(S) masked, O = P@V / rowsum
    y_core = (O_heads @ Wo[rows, :])^T          -> [768, 2048] partial
Host: out[b] = (y[2b] + y[2b+1])^T + bo + bv @ Wo   (attn rows sum to 1,
so the V bias contributes exactly bv @ Wo_rows to every output row).

Layout trick: everything is kept transposed (feature dim on partitions) so
every matmul has a 512-wide moving operand and can run at full PE rate in
float32r (x itself arrives host-transposed, so no on-chip transposes at
all). Softmax runs without max-subtraction (logits are O(1) here), with
row sums computed by an all-ones-matrix matmul in the same transposed
layout (fused_denb: the [128,128] ones stationary replicates the sum to
all partitions, so no broadcast matmul is needed), and the reciprocal via
the fast approximate DVE op (the exact InstReciprocal costs ~4us/tile on
HW and was worth ~48us of wall clock).
Projections run one 512-column chunk ahead of attention because the
tril(k=1) mask lets each query attend one token into the future.
Diagonal-band blocks restrict S/exp/mask/PV/rowsum to the valid column
range (everything below 128j-2 is structurally masked, with the j=3 slice
widened to 256 because f32r matmuls below 256-wide run at quarter rate).
Masking is in-place gpsimd affine_select on ~128-wide strips (DVE mask
multiplies measured ~19us slower on HW). x/y/weight DMAs are batched into
few multi-d descriptors, split 2-ways so two DMA engines run in parallel.
"""

import math
from contextlib import ExitStack

import numpy as np

import concourse.tile as tile
from concourse import bacc, mybir
from concourse.bass_utils import run_bass_kernel_spmd

F32 = mybir.dt.float32
F32R = mybir.dt.float32r
BF16 = mybir.dt.bfloat16
AF = mybir.ActivationFunctionType

B, L, D, H = 4, 2048, 768, 6
HD = 128           # head dim
HPC = 3            # heads per core
DH = HPC * HD      # 384: per-core projection width
NCORES = 8
P = 128
CHUNK = 512        # q-chunk width (moving-operand size)
NCHUNK = L // CHUNK
LT = L // P        # 16 L-tiles
DT = D // P        # 6 d-tiles
SCALE = 1.0 / math.sqrt(HD)

_cache = {}


def build_nc(mm_fast=True, enable_asserts=False, reps=1,
             mm1_bufs=3, aux_bufs=1, pb_in_mm1=False, den_on_dve=False,
             est_bufs=6, attn_mode="seq", mask_gpsimd=False, vt_on_act=False,
             fused_denb=True, qk_on_dve=False, yst_eng="v",
             attn_order="pipeline", qt_bufs=2, denom_mode="mm",
             mask_mode="select", pv_bufs=2, esum_ways=1, esum_eng="vvv",
             xt_bufs=2, kv_parity=False, recip_fast=True, xt_split=2,
             y_split=2, mm_bf16=False):
    nc = bacc.Bacc(
        "TRN2",
        target_bir_lowering=False,
        debug=False,
        enable_asserts=enable_asserts,
        num_devices=NCORES,
    )
    MMDT = BF16 if mm_bf16 else (F32R if mm_fast else F32)
    IODT = BF16 if mm_bf16 else F32
    x_d = nc.dram_tensor("x", [D, L], IODT, kind="ExternalInput").ap()
    wq_d = nc.dram_tensor("wq", [D, DH], IODT, kind="ExternalInput").ap()
    wk_d = nc.dram_tensor("wk", [D, DH], IODT, kind="ExternalInput").ap()
    wv_d = nc.dram_tensor("wv", [D, DH], IODT, kind="ExternalInput").ap()
    wo_d = nc.dram_tensor("wo", [DH, D], IODT, kind="ExternalInput").ap()
    bq_d = nc.dram_tensor("bq", [DH], F32, kind="ExternalInput").ap()
    bk_d = nc.dram_tensor("bk", [DH], F32, kind="ExternalInput").ap()
    y_d = nc.dram_tensor("y", [D, L], F32, kind="ExternalOutput").ap()
    cast = lambda ap: ap  # noqa: E731

    with tile.TileContext(nc) as tc, ExitStack() as ctx:
        if mm_bf16:
            ctx.enter_context(nc.allow_low_precision(
                "bf16 matmul operands; all accumulation stays in fp32 PSUM"))
        const = ctx.enter_context(tc.tile_pool(name="const", bufs=1))
        wpool = ctx.enter_context(tc.tile_pool(name="wts", bufs=1))
        kvpool = ctx.enter_context(tc.tile_pool(name="kv", bufs=1))
        xtpool = ctx.enter_context(tc.tile_pool(name="xt", bufs=xt_bufs))
        qpool = ctx.enter_context(tc.tile_pool(name="qt", bufs=qt_bufs))
        estpool = ctx.enter_context(tc.tile_pool(name="est", bufs=est_bufs))
        smpool = ctx.enter_context(tc.tile_pool(name="sm", bufs=2))
        opool = ctx.enter_context(tc.tile_pool(name="ot", bufs=2))
        espool = ctx.enter_context(tc.tile_pool(name="esum", bufs=2))
        ypool = ctx.enter_context(tc.tile_pool(name="yst", bufs=2))
        ps_mm = ctx.enter_context(tc.tile_pool(name="psmm", bufs=mm1_bufs, space="PSUM"))
        ps_acc = ctx.enter_context(tc.tile_pool(name="psacc", bufs=2, space="PSUM"))
        if attn_mode != "pair":
            ps_aux = ctx.enter_context(
                tc.tile_pool(name="psaux", bufs=aux_bufs, space="PSUM"))

        ones_f32 = const.tile([P, 1], F32, tag="ones_f32", name="ones_f32")
        nc.vector.memset(ones_f32[:], 1.0)
        ones_col = const.tile([P, 1], MMDT, tag="ones_col", name="ones_col")
        nc.scalar.copy(ones_col[:], ones_f32[:])
        ones_mf = const.tile([P, P], F32, tag="ones_mf", name="ones_mf")
        nc.vector.memset(ones_mf[:], 1.0)
        ones_mat = const.tile([P, P], MMDT, tag="ones_mat", name="ones_mat")
        nc.scalar.copy(ones_mat[:], ones_mf[:])
        ones_rf = const.tile([1, P], F32, tag="ones_rf", name="ones_rf")
        nc.vector.memset(ones_rf[:], 1.0)
        ones_row = const.tile([1, P], MMDT, tag="ones_row", name="ones_row")
        nc.scalar.copy(ones_row[:], ones_rf[:])
        # 5 diagonal-band masks (0/1), shared by all chunks/heads.
        # mask[j][kp, qq] = 1 iff kp - qq <= 1 - 128*j
        masks = []
        if mask_mode == "mul":
            for j in range(5):
                mj = const.tile([P, CHUNK], F32, tag=f"mask{j}", name=f"mask{j}")
                nc.gpsimd.memset(mj[:], 1.0)
                # keep (mask=1) where kp - qq <= 1 - 128j, i.e. qq - kp + (1-128j) >= 0
                nc.gpsimd.affine_select(
                    out=mj[:],
                    in_=mj[:],
                    pattern=[[1, CHUNK]],
                    compare_op=mybir.AluOpType.is_ge,
                    fill=0.0,
                    base=1 - 128 * j,
                    channel_multiplier=-1,
                )
                masks.append(mj)

        # Per-diagonal column starts: matmul slice (MS0), exp slice (ES0),
        # and affine-select range (SEL). f32r widens the j=3 matmul to 256
        # (quarter rate below 256-wide); bf16 has no such penalty but needs
        # 8-element (16B) aligned starts.
        if mm_bf16:
            MS0 = {0: 0, 1: 120, 2: 248, 3: 376, 4: 504}
            ES0 = MS0
            SEL = {0: (0, 128), 1: (120, 256), 2: (248, 384), 3: (376, CHUNK),
                   4: (504, CHUNK)}
        else:
            MS0 = {0: 0, 1: 126, 2: 254, 3: 256, 4: 510}
            ES0 = {0: 0, 1: 126, 2: 254, 3: 382, 4: 510}
            SEL = {0: (0, 128), 1: (126, 254), 2: (254, 382), 3: (256, CHUNK),
                   4: (510, CHUNK)}

        def apply_mask(est, j, sl):
            # zero est where kp - qq > 1 - 128j. Only columns in [m0, m1) can
            # be invalid (the diagonal triangle); outside that, all kp valid.
            # The select also zero-fills any junk between the matmul slice
            # start and the exp slice start.
            if mask_mode == "mul":
                eng = nc.gpsimd if mask_gpsimd else nc.vector
                eng.tensor_mul(est[:, sl], est[:, sl], masks[j][:, sl])
                return
            m0, m1 = SEL[j]
            nc.gpsimd.affine_select(
                out=est[:, m0:m1],
                in_=est[:, m0:m1],
                pattern=[[1, m1 - m0]],
                compare_op=mybir.AluOpType.is_ge,
                fill=0.0,
                base=m0 + 1 - 128 * j,
                channel_multiplier=-1,
            )

        x_3d = x_d.rearrange("(d p) l -> p d l", p=P)
        y_3d = y_d.rearrange("(d p) l -> p d l", p=P)

        # denominators are sums of exps in [~1e-3, ~1e4]: far from the
        # approx-reciprocal edge cases, and 18 correct bits is plenty for
        # softmax normalization.
        recip_fn = (nc.vector.reciprocal_approx_fast if recip_fast
                    else nc.vector.reciprocal)

        def make_xT(c):
            # xT columns: block d lives at [d*CHUNK, (d+1)*CHUNK).
            # xt_split>1 splits the chunk DMA so multiple DMA engines run in
            # parallel (one descriptor chain executes on a single engine).
            xT = xtpool.tile([P, DT * CHUNK], MMDT, tag="xT", name="xT")
            dper = DT // xt_split
            for i in range(xt_split):
                d0 = i * dper
                nc.sync.dma_start(
                    out=xT[:, d0 * CHUNK:(d0 + dper) * CHUNK].rearrange(
                        "p (d c) -> p d c", c=CHUNK),
                    in_=x_3d[:, d0:d0 + dper,
                             c * CHUNK:(c + 1) * CHUNK].bitcast(MMDT),
                )
            return xT

        # DMA issue order: wq and chunk-0 x columns first so the first
        # projection matmuls can start early; wk/wv next; wo/biases later.
        # One batched DMA per weight tensor (d-tiles side by side).
        def load_w(w3, dram, dt_, width):
            nc.sync.dma_start(
                out=w3[:].rearrange("p (d m) -> p d m", m=width),
                in_=dram.rearrange("(d p) m -> p d m", p=P).bitcast(dt_),
            )

        wq_all = wpool.tile([P, DT * DH], MMDT, tag="wq", name="wq")
        load_w(wq_all, wq_d, MMDT, DH)

        xT0 = make_xT(0)
        wk_all = wpool.tile([P, DT * DH], MMDT, tag="wk", name="wk")
        load_w(wk_all, wk_d, MMDT, DH)

        wv_all = wpool.tile([P, DT * DH], MMDT, tag="wv", name="wv")
        load_w(wv_all, wv_d, MMDT, DH)

        bq_all = wpool.tile([P, HPC], F32, tag="bq", name="bq")
        nc.sync.dma_start(
            out=bq_all[:], in_=bq_d.rearrange("(h p) -> p h", p=P))
        bk_all = wpool.tile([P, HPC], F32, tag="bk", name="bk")
        nc.sync.dma_start(
            out=bk_all[:], in_=bk_d.rearrange("(h p) -> p h", p=P))

        wo_all = wpool.tile([P, HPC * D], MMDT, tag="wo", name="wo")
        load_w(wo_all, wo_d, MMDT, D)


        # K^T per head [hd=128, L]; V per L-tile [kpos=128, 3*hd].
        # Two parity sets so rep r+1's K/V projection writes don't serialize
        # behind rep r's tail-chunk attention reads (kv_parity pingpong).
        nkv = 2 if kv_parity and reps > 1 else 1
        kT_sets = [[kvpool.tile([P, L], MMDT, tag=f"kT{h}_{s}", name=f"kT{h}_{s}")
                    for h in range(HPC)] for s in range(nkv)]
        vt_sets = [[kvpool.tile([P, DH], MMDT, tag=f"v{t}_{s}", name=f"v{t}_{s}")
                    for t in range(LT)] for s in range(nkv)]
        kT = kT_sets[0]
        vt = vt_sets[0]

        def proj_chunk(c, xT=None):
            # ---- x^T columns for this chunk (x arrives host-transposed) ----
            if xT is None:
                xT = make_xT(c)

            # ---- Q^T, K^T projections for this chunk ----
            qT = [qpool.tile([P, CHUNK], MMDT, tag=f"qT{h}", name=f"qT{h}")
                  for h in range(HPC)]
            for h in range(HPC):
                pq = ps_acc.tile([P, CHUNK], F32, tag="acc", name="acc")
                for d in range(DT):
                    nc.tensor.matmul(
                        pq[:],
                        cast(wq_all[:, d * DH + h * P:d * DH + (h + 1) * P]),
                        cast(xT[:, d * CHUNK:(d + 1) * CHUNK]),
                        start=(d == 0),
                        stop=(d == DT - 1),
                    )
                if qk_on_dve:
                    nc.vector.tensor_scalar_add(qT[h][:], pq[:], bq_all[:, h:h + 1])
                else:
                    nc.scalar.activation(qT[h][:], pq[:], AF.Identity,
                                         bias=bq_all[:, h:h + 1])
                pk = ps_acc.tile([P, CHUNK], F32, tag="acc", name="acc")
                for d in range(DT):
                    nc.tensor.matmul(
                        pk[:],
                        cast(wk_all[:, d * DH + h * P:d * DH + (h + 1) * P]),
                        cast(xT[:, d * CHUNK:(d + 1) * CHUNK]),
                        start=(d == 0),
                        stop=(d == DT - 1),
                    )
                if qk_on_dve:
                    nc.vector.tensor_scalar_add(
                        kT[h][:, c * CHUNK:(c + 1) * CHUNK], pk[:], bk_all[:, h:h + 1]
                    )
                else:
                    nc.scalar.activation(
                        kT[h][:, c * CHUNK:(c + 1) * CHUNK], pk[:], AF.Identity,
                        bias=bk_all[:, h:h + 1],
                    )

            # ---- V projection (natural layout) ----
            for i in range(CHUNK // P):
                t = c * (CHUNK // P) + i
                pv = ps_acc.tile([P, DH], F32, tag="acc", name="acc")
                for d in range(DT):
                    nc.tensor.matmul(
                        pv[:],
                        cast(xT[:, d * CHUNK + i * P: d * CHUNK + (i + 1) * P]),
                        cast(wv_all[:, d * DH:(d + 1) * DH]),
                        start=(d == 0),
                        stop=(d == DT - 1),
                    )
                if vt_on_act:
                    nc.scalar.copy(vt[t][:], pv[:])
                else:
                    nc.vector.tensor_copy(vt[t][:], pv[:])
            return qT

        def attn_chunk_ileave(c, qT):
            # ---- attention, 3 heads interleaved per kb block ----
            # Denominators for all heads pack into one PSUM bank (rows
            # 0/32/64 -- tile_position requires 32-aligned output rows).
            # A single start=True (h0,kb0) clears the bank; the other heads'
            # first writes overwrite via the has_written bits.
            KB = 4 * c + 5 if c < NCHUNK - 1 else LT
            oTn = [opool.tile([P, CHUNK], MMDT, tag=f"oT{h}", name=f"oT{h}")
                   for h in range(HPC)]
            po = [ps_acc.tile([P, CHUNK], F32, tag=f"pv{h}", name=f"pv{h}", bufs=1)
                  for h in range(HPC)]
            pdall = ps_aux.tile([P, CHUNK], F32, tag="aux", name="aux", bufs=1)
            nc.vector.memset(pdall[:], 0.0)
            for kb in range(KB):
                j = kb - 4 * c
                for h in range(HPC):
                    pst = ps_mm.tile([P, CHUNK], F32, tag="mm1", name="mm1")
                    nc.tensor.matmul(
                        pst[:],
                        cast(kT[h][:, kb * P:(kb + 1) * P]),
                        cast(qT[h][:]),
                        start=True,
                        stop=True,
                    )
                    est = estpool.tile([P, CHUNK], MMDT, tag="est", name="est")
                    nc.scalar.activation(est[:], pst[:], AF.Exp, scale=SCALE)
                    if j >= 0:
                        apply_mask(est, j, slice(0, CHUNK))
                    nc.tensor.matmul(
                        po[h][:],
                        cast(vt[kb][:, h * P:(h + 1) * P]),
                        cast(est[:]),
                        start=(kb == 0),
                        stop=(kb == KB - 1),
                    )
                    # All three heads' row sums accumulate into one PSUM bank
                    # (rows 0/32/64). The bank is DVE-memset to zero up front,
                    # so plain accumulation (never start=True) is correct on
                    # both hardware and sim regardless of has_written state.
                    nc.tensor.matmul(
                        pdall[32 * h:32 * h + 1, :],
                        cast(ones_col[:]),
                        cast(est[:]),
                        start=False,
                        stop=(kb == KB - 1 and h == HPC - 1),
                        skip_group_check=True,
                    )
            for h in range(HPC):
                den_sb = smpool.tile([1, CHUNK], MMDT, tag=f"den{h}", name=f"den{h}")
                nc.scalar.copy(den_sb[:], pdall[32 * h:32 * h + 1, :])
                pb = ps_mm.tile([P, CHUNK], F32, tag="mm1", name="mm1")
                nc.tensor.matmul(
                    pb[:], cast(ones_row[:]), cast(den_sb[:]), start=True, stop=True
                )
                recip = smpool.tile([P, CHUNK], F32, tag=f"recip{h}", name=f"recip{h}")
                nc.vector.reciprocal(recip[:], pb[:])
                nc.vector.tensor_mul(oTn[h][:], po[h][:], recip[:])
            outproj_chunk(c, oTn)

        def attn_chunk(c, qT):
            # ---- attention for this q-chunk ----
            KB = 4 * c + 5 if c < NCHUNK - 1 else LT
            oTn = [opool.tile([P, CHUNK], MMDT, tag=f"oT{h}", name=f"oT{h}")
                   for h in range(HPC)]
            for h in range(HPC):
                po = ps_acc.tile([P, CHUNK], F32, tag="pvacc", name="pvacc",
                                 bufs=pv_bufs)
                esum = None
                if denom_mode == "esum":
                    # accumulate exp tiles elementwise (DVE or gpsimd per
                    # head, esum_ways-way tree to cut chain depth); the
                    # ones-matmuls at the end replicate the row sums to all
                    # partitions (replaces one PE stream per kb block)
                    eng = {"v": nc.vector, "g": nc.gpsimd}[esum_eng[h]]
                    esum = [espool.tile([P, CHUNK], MMDT, tag=f"esum{w}",
                                        name=f"esum{w}")
                            for w in range(esum_ways)]
                    esum_started = [False] * esum_ways
                    way_s0 = [0] * esum_ways
                    pd = ps_aux.tile([P, CHUNK], F32, tag="aux", name="aux")
                else:
                    pd = ps_aux.tile([P if fused_denb else 1, CHUNK], F32,
                                     tag="aux", name="aux")
                for kb in range(KB):
                    # Diagonal-band blocks (j >= 1): every column below
                    # 128j-1 is fully masked, so restrict all ops to the
                    # valid column range (8B-aligned start). The skipped
                    # region of est is stale but never read. In select mode
                    # the j=3 matmul slice is widened to 256 (f32r runs at
                    # quarter rate below 256-wide); the select zero-fills the
                    # never-exp'd [256, 382) region so PV/rowsum stay exact.
                    j = kb - 4 * c
                    if j >= 1 and mask_mode != "select":
                        s0 = e0 = 128 * j - 2
                    else:
                        s0 = MS0[j] if j >= 0 else 0
                        e0 = ES0[j] if j >= 0 else 0
                    sl = slice(s0, CHUNK)
                    esl = slice(e0, CHUNK)
                    pst = ps_mm.tile([P, CHUNK], F32, tag="mm1", name="mm1")
                    nc.tensor.matmul(
                        pst[:, sl],
                        cast(kT[h][:, kb * P:(kb + 1) * P]),
                        cast(qT[h][:, sl]),
                        start=True,
                        stop=True,
                    )
                    est = estpool.tile([P, CHUNK], MMDT, tag="est", name="est")
                    nc.scalar.activation(est[:, esl], pst[:, esl], AF.Exp,
                                         scale=SCALE)
                    if j >= 0:
                        apply_mask(est, j, sl)
                    nc.tensor.matmul(
                        po[:, sl],
                        cast(vt[kb][:, h * P:(h + 1) * P]),
                        cast(est[:, sl]),
                        start=(kb == 0),
                        stop=(kb == KB - 1),
                    )
                    if denom_mode == "esum":
                        w = kb % esum_ways
                        if not esum_started[w]:
                            # first write per way covers the full remaining
                            # column range (sl ranges narrow as kb grows) so
                            # later adds always land on initialized data
                            nc.vector.tensor_copy(esum[w][:, sl], est[:, sl])
                            esum_started[w] = True
                            way_s0[w] = s0
                        else:
                            eng.tensor_add(
                                esum[w][:, sl], esum[w][:, sl], est[:, sl]
                            )
                    else:
                        # row-sum accumulation; fused_denb replicates the sum
                        # to all 128 partitions (ones matrix) so no broadcast
                        # matmul is needed afterwards
                        nc.tensor.matmul(
                            pd[:, sl],
                            cast(ones_mat[:] if fused_denb else ones_col[:]),
                            cast(est[:, sl]),
                            start=(kb == 0),
                            stop=(kb == KB - 1),
                        )
                # normalize: oTn = po * (1 / rowsum) broadcast over partitions
                recip = smpool.tile([P, CHUNK], F32, tag="recip", name="recip")
                if denom_mode == "esum":
                    nways = sum(esum_started)
                    wlast = nways - 1
                    for w in range(nways):
                        wsl = slice(way_s0[w], CHUNK)
                        nc.tensor.matmul(
                            pd[:, wsl], cast(ones_mat[:]), cast(esum[w][:, wsl]),
                            start=(w == 0), stop=(w == wlast),
                        )
                    recip_fn(recip[:], pd[:])
                elif fused_denb:
                    recip_fn(recip[:], pd[:])
                else:
                    den_sb = smpool.tile([1, CHUNK], MMDT, tag="den", name="den")
                    if den_on_dve:
                        nc.vector.tensor_copy(den_sb[:], pd[:])
                    else:
                        nc.scalar.copy(den_sb[:], pd[:])
                    if pb_in_mm1:
                        pb = ps_mm.tile([P, CHUNK], F32, tag="mm1", name="mm1")
                    else:
                        pb = ps_aux.tile([P, CHUNK], F32, tag="aux", name="aux")
                    nc.tensor.matmul(
                        pb[:], cast(ones_row[:]), cast(den_sb[:]),
                        start=True, stop=True,
                    )
                    nc.vector.reciprocal(recip[:], pb[:])
                nc.vector.tensor_mul(oTn[h][:], po[:], recip[:])
            outproj_chunk(c, oTn)

        def attn_chunk_pair(c, qT):
            # Paired kb blocks: one [P, 2*CHUNK] score-PSUM (2 banks) and one
            # exp per pair (halves ACT instruction count + sem traffic).
            # Masking via in-place gpsimd affine_select per diagonal half
            # (also zeroes the never-computed junk region of partial halves).
            # Denominators via esum accumulation on DVE/gpsimd (no PE rowsum
            # matmuls) + per-way ones matmuls at the end of each head.
            # The 2-wide lookahead block (j=4) rides in the last pair's
            # unused leading columns.
            KBP = 4 * c + 4 if c < NCHUNK - 1 else LT
            has_j4 = c < NCHUNK - 1
            npairs = KBP // 2
            kb4 = 4 * c + 4
            # S/PV matmul column starts per diagonal j (j=3 widened 382->256
            # to keep the f32r moving width >= 256), and affine-select spans.
            mm_s0 = {0: 0, 1: 126, 2: 254, 3: 256}
            sel_end = {0: 128, 1: 256, 2: 384, 3: 512}
            oTn = [opool.tile([P, CHUNK], MMDT, tag=f"oT{h}", name=f"oT{h}")
                   for h in range(HPC)]
            for h in range(HPC):
                po = ps_acc.tile([P, CHUNK], F32, tag="pvacc", name="pvacc",
                                 bufs=pv_bufs)
                eng = {"v": nc.vector, "g": nc.gpsimd}[esum_eng[h]]
                esum = [espool.tile([P, CHUNK], MMDT, tag=f"esum{w}",
                                    name=f"esum{w}") for w in range(esum_ways)]
                esum_started = [False] * esum_ways
                way_s0 = [0] * esum_ways

                def esum_add(kb, src, dst_sl, src_sl):
                    w = kb % esum_ways
                    if not esum_started[w]:
                        # first write per way has the widest range (ranges
                        # narrow as kb grows), so later adds land on
                        # initialized data
                        eng.tensor_copy(esum[w][:, dst_sl], src[:, src_sl])
                        esum_started[w] = True
                        way_s0[w] = dst_sl.start
                    else:
                        eng.tensor_add(esum[w][:, dst_sl], esum[w][:, dst_sl],
                                       src[:, src_sl])

                for p in range(npairs):
                    last_pair = p == npairs - 1
                    pair = ps_mm.tile([P, 2 * CHUNK], F32, tag="mm1", name="mm1")
                    est = estpool.tile([P, 2 * CHUNK], MMDT, tag="est", name="est")
                    js = []
                    for half in (0, 1):
                        kb = 2 * p + half
                        j = kb - 4 * c
                        s0 = mm_s0[j] if j >= 0 else 0
                        js.append((half, kb, j, s0))
                        nc.tensor.matmul(
                            pair[:, half * CHUNK + s0:(half + 1) * CHUNK],
                            cast(kT[h][:, kb * P:(kb + 1) * P]),
                            cast(qT[h][:, s0:CHUNK]),
                            start=True,
                            stop=True,
                        )
                    if has_j4 and last_pair:
                        nc.tensor.matmul(
                            pair[:, 0:2],
                            cast(kT[h][:, kb4 * P:(kb4 + 1) * P]),
                            cast(qT[h][:, CHUNK - 2:CHUNK]),
                            start=True,
                            stop=True,
                            skip_group_check=True,
                        )
                    x0 = js[0][3]
                    nc.scalar.activation(est[:, x0:2 * CHUNK],
                                         pair[:, x0:2 * CHUNK], AF.Exp,
                                         scale=SCALE)
                    for half, kb, j, s0 in js:
                        if j >= 0:
                            e0 = half * CHUNK
                            se = sel_end[j]
                            nc.gpsimd.affine_select(
                                out=est[:, e0:e0 + se],
                                in_=est[:, e0:e0 + se],
                                pattern=[[1, se]],
                                compare_op=mybir.AluOpType.is_ge,
                                fill=0.0,
                                base=1 - 128 * j,
                                channel_multiplier=-1,
                            )
                    if has_j4 and last_pair:
                        # j2's select zeroed est[:, 0:2]; overwrite with the
                        # lookahead exp, then mask it (only (kp=0, qq=511)
                        # survives)
                        nc.scalar.activation(est[:, 0:2], pair[:, 0:2],
                                             AF.Exp, scale=SCALE)
                        nc.gpsimd.affine_select(
                            out=est[:, 0:2],
                            in_=est[:, 0:2],
                            pattern=[[1, 2]],
                            compare_op=mybir.AluOpType.is_ge,
                            fill=0.0,
                            base=-1,
                            channel_multiplier=-1,
                        )
                    for half, kb, j, s0 in js:
                        sl = slice(s0, CHUNK)
                        nc.tensor.matmul(
                            po[:, sl],
                            cast(vt[kb][:, h * P:(h + 1) * P]),
                            cast(est[:, half * CHUNK + s0:(half + 1) * CHUNK]),
                            start=(p == 0 and half == 0),
                            stop=(last_pair and half == 1 and not has_j4),
                        )
                        esum_add(kb, est, sl,
                                 slice(half * CHUNK + s0, (half + 1) * CHUNK))
                    if has_j4 and last_pair:
                        nc.tensor.matmul(
                            po[:, CHUNK - 2:CHUNK],
                            cast(vt[kb4][:, h * P:(h + 1) * P]),
                            cast(est[:, 0:2]),
                            start=False,
                            stop=True,
                        )
                        esum_add(kb4, est, slice(CHUNK - 2, CHUNK), slice(0, 2))

                pd = ps_mm.tile([P, 2 * CHUNK], F32, tag="mm1", name="mm1")
                nways = sum(esum_started)
                for w in range(nways):
                    wsl = slice(way_s0[w], CHUNK)
                    nc.tensor.matmul(
                        pd[:, wsl], cast(ones_mat[:]), cast(esum[w][:, wsl]),
                        start=(w == 0), stop=(w == nways - 1),
                    )
                recip = smpool.tile([P, CHUNK], F32, tag="recip", name="recip")
                nc.vector.reciprocal(recip[:], pd[:, 0:CHUNK])
                nc.vector.tensor_mul(oTn[h][:], po[:], recip[:])
            outproj_chunk(c, oTn)

        def outproj_chunk(c, oTn):
            # ---- output projection for this chunk (one batched y-DMA) ----
            ybig = ypool.tile([P, DT * CHUNK], F32, tag="yst", name="yst")
            for do in range(DT):
                py = ps_acc.tile([P, CHUNK], F32, tag="acc", name="acc")
                for h in range(HPC):
                    nc.tensor.matmul(
                        py[:],
                        cast(wo_all[:, h * D + do * P:h * D + (do + 1) * P]),
                        cast(oTn[h][:]),
                        start=(h == 0),
                        stop=(h == HPC - 1),
                    )
                ysl = ybig[:, do * CHUNK:(do + 1) * CHUNK]
                if yst_eng == "a":
                    nc.scalar.copy(ysl, py[:])
                elif yst_eng == "g":
                    nc.gpsimd.tensor_copy(ysl, py[:])
                else:
                    nc.vector.tensor_copy(ysl, py[:])
            dper = DT // y_split
            for i in range(y_split):
                d0 = i * dper
                nc.sync.dma_start(
                    out=y_3d[:, d0:d0 + dper, c * CHUNK:(c + 1) * CHUNK],
                    in_=ybig[:, d0 * CHUNK:(d0 + dper) * CHUNK].rearrange(
                        "p (d c) -> p d c", c=CHUNK),
                )

        # Pipeline: attention of chunk c needs K/V through block 4c+4, which
        # lives in chunk c+1's rows (the tril(k=1) one-token lookahead). So
        # run projections one chunk ahead of attention. reps>1 repeats the
        # whole compute for benchmarking (amortizes dispatch overhead).
        attn = {"ileave": attn_chunk_ileave, "pair": attn_chunk_pair,
                "seq": attn_chunk}[attn_mode]
        for _rep in range(reps):
            kT = kT_sets[_rep % nkv]
            vt = vt_sets[_rep % nkv]
            qTs = {}
            qTs[0] = proj_chunk(0, xT=xT0 if _rep == 0 else None)
            if attn_order == "small_last":
                # attn(c) only needs proj(c+1); run the smallest chunk (0)
                # last so the un-overlapped kernel tail is as short as
                # possible. Needs qT(0) alive until the end (qpool bufs).
                qTs[1] = proj_chunk(1)
                qTs[2] = proj_chunk(2)
                attn(1, qTs.pop(1))
                qTs[3] = proj_chunk(3)
                attn(2, qTs.pop(2))
                attn(3, qTs.pop(3))
                attn(0, qTs.pop(0))
            else:
                for c in range(1, NCHUNK):
                    qTs[c] = proj_chunk(c)
                    attn(c - 1, qTs.pop(c - 1))
                attn(NCHUNK - 1, qTs.pop(NCHUNK - 1))

    nc.compile()
    return nc


def shard_inputs(x, Wq, bq, Wk, bk, Wv, bv, Wo, bo, mm_bf16=False):
    import ml_dtypes

    iodt = ml_dtypes.bfloat16 if mm_bf16 else np.float32
    x = np.asarray(x, dtype=np.float32)
    in_maps = []
    for core in range(NCORES):
        b = core // 2
        g = core % 2
        sl = slice(g * DH, (g + 1) * DH)
        in_maps.append({
            "x": np.ascontiguousarray(x[b].T.astype(iodt)),
            "wq": np.ascontiguousarray(np.asarray(Wq, np.float32)[:, sl].astype(iodt)),
            "wk": np.ascontiguousarray(np.asarray(Wk, np.float32)[:, sl].astype(iodt)),
            "wv": np.ascontiguousarray(np.asarray(Wv, np.float32)[:, sl].astype(iodt)),
            "wo": np.ascontiguousarray(np.asarray(Wo, np.float32)[sl, :].astype(iodt)),
            "bq": np.ascontiguousarray(np.asarray(bq, np.float32)[sl]),
            "bk": np.ascontiguousarray(np.asarray(bk, np.float32)[sl]),
        })
    return in_maps


def unshard_output(results, Wo, bv, bo):
    out = np.empty((B, L, D), dtype=np.float32)
    for b in range(B):
        acc = results[2 * b]["y"] + results[2 * b + 1]["y"]  # [D, L]
        out[b] = acc.T
    corr = np.asarray(bo, np.float32) + np.asarray(bv, np.float32) @ np.asarray(
        Wo, np.float32
    )
    out += corr
    return out


def run(inputs, trace=False, **kw):
    if "nc" not in _cache:
        _cache["nc"] = build_nc()
    nc = _cache["nc"]
    in_maps = shard_inputs(**inputs)
    res = run_bass_kernel_spmd(nc, in_maps, list(range(NCORES)), trace=trace, **kw)
    out = unshard_output(res.results, inputs["Wo"], inputs["bv"], inputs["bo"])
    return out, res


def kernel(**inputs):
    out, _ = run(inputs)
    return out

